# revision 12
# baseline (speedup 1.0000x reference)
"""Fp8 per-token/per-channel quantized linear for Trainium2, 8 NeuronCores.

Computation (matches the jax reference):
    amax[m]  = max_k |x[m, k]|                       (x is bf16)
    xs[m]    = max(amax, 1e-10) / 448
    x_q      = e4m3fn_round(x / xs)                  (values up to +-448)
    out      = bf16((x_q @ W^T) * xs * w_scales) + bf16(bias)

Mapping to TRN2 hardware:
  * TRN's fp8 E4M3 saturates at +-240 (256..448 are Inf/NaN), so we quantize
    at HALF scale: x_q' = e4m3_round(x * (224/amax)) == x_q / 2 exactly (the
    fp8 grid is self-similar under powers of two), and fold the factor 2 into
    the output scale: out = psum * (amax/224) * w_scales.  The reference
    weights are already exactly fp8-representable, so casting them is lossless.
  * Sharding: row-parallel over M (8 cores x 1024 rows).  Each core quantizes
    only its own rows, and streams the full weight, transposed on host to
    [K, N] tile layout and losslessly re-encoded to fp8.
  * x_q is transposed on-chip into [K, M] layout with PE transpose matmuls
    (contraction must sit on partitions for both matmul operands).
  * Main GEMM runs in fp8 with perf_mode=DoubleRow (k=256 per matmul).

Schedule (v4): the kernel is PE-bound (DoubleRow GEMM ~221us + transposes
~25us).  The per-tile producer chain is DVE amax (f32 reduce, no fast DVE
mode exists: ~4.6us) + tiny scale chain, ACT quant copy (~3.7us) and two
ACT psum evicts (~4.1us); DVE additionally runs the fused epilogue
(psum*xs*ws in one scalar_tensor_tensor).  ACT (~7.9us) and DVE (~6.6us)
both stay under the PE's 8.6us per-tile consumption of T(mt) + GEMM(mt,0)
+ GEMM(mt,1), so phase 1 runs PE-bound; phase 2 is pure GEMM.  DMA queue
fairness is round-robin per ~descriptor, so x tiles load as single [128,4096]
DMAs (8KB runs) on the sync ring while weight slabs ride the scalar ring,
the first two split into 4KB-run quarters (finer deps + fairness).
"""

import os
import numpy as np
import ml_dtypes
from contextlib import ExitStack

import concourse.bass as bass
import concourse.bacc as bacc
import concourse.tile as tile
from concourse import mybir
from concourse.bass_utils import run_bass_kernel_spmd
from concourse.masks import make_identity

P = 128
M, K, N = 8192, 4096, 4096
NCORES = 8
M_SHARD = M // NCORES          # 1024 rows of x per core
M_TILES = M_SHARD // P         # 8
K_SUBS = K // P                # 32
K_SUPERS = K // (2 * P)        # 16 (DoubleRow consumes 256 rows of K)
KH = K // 2                    # 2048, half-tile for split reduces
N_BLK = 512
N_BLKS = N // N_BLK            # 8
NB_PHASE1 = 2                  # GEMM N-blocks interleaved into the quant loop
N_QUARTERED = 2                # leading slabs loaded as 4 quarter-DMAs

FP8 = mybir.dt.float8e4
F32 = mybir.dt.float32
BF16 = mybir.dt.bfloat16

USE_IS_TRANSPOSE = True

_PROGRAM_CACHE = {}


def _build_program():
    nc = bacc.Bacc(None, target_bir_lowering=False)

    x_d = nc.declare_dram_parameter("x", [M_SHARD, K], BF16, isOutput=False)
    # host layout: wt[nb, p, ksub, n] = weight[nb*512 + n, ksub*128 + p],
    # losslessly re-encoded to fp8 (reference weights are fp8-round-tripped)
    wt_d = nc.declare_dram_parameter("wt", [N_BLKS, P, K_SUBS, N_BLK], FP8, isOutput=False)
    ws_d = nc.declare_dram_parameter("ws", [N], F32, isOutput=False)
    bias_d = nc.declare_dram_parameter("bias", [N], F32, isOutput=False)
    out_d = nc.declare_dram_parameter("out", [M_SHARD, N], BF16, isOutput=True)

    x_ap = x_d[:]
    wt_ap = wt_d[:]
    out_ap = out_d[:]

    with tile.TileContext(nc) as tc, ExitStack() as ctx:
        singles = ctx.enter_context(tc.tile_pool(name="singles", bufs=1))
        xpool = ctx.enter_context(tc.tile_pool(name="xpool", bufs=3))
        xqpool = ctx.enter_context(tc.tile_pool(name="xqpool", bufs=2))
        stats = ctx.enter_context(tc.tile_pool(name="stats", bufs=4))
        xspool = ctx.enter_context(tc.tile_pool(name="xspool", bufs=M_TILES))
        xqtpool = ctx.enter_context(tc.tile_pool(name="xqtpool", bufs=M_TILES))
        wqpool = ctx.enter_context(tc.tile_pool(name="wqpool", bufs=4 * N_QUARTERED))
        wpool = ctx.enter_context(tc.tile_pool(name="wpool", bufs=3))
        opool = ctx.enter_context(tc.tile_pool(name="opool", bufs=4))
        psum_tr = ctx.enter_context(tc.tile_pool(name="psum_tr", bufs=2, space="PSUM"))
        psum_mm = ctx.enter_context(tc.tile_pool(name="psum_mm", bufs=4, space="PSUM"))

        # ---- upfront DMA issue: x tiles 0-1 on the sync ring; weight slabs
        # on the scalar ring (first two quartered); ws/bias broadcasts are
        # HBM-read-light and use the scalar ring's broadcast path.
        x_tiles = [None] * M_TILES

        def issue_x(mt):
            t = xpool.tile([P, K], BF16, tag="xt")
            nc.sync.dma_start(out=t[:], in_=x_ap[mt * P:(mt + 1) * P, :])
            x_tiles[mt] = t

        wslab_tiles = [None] * N_BLKS

        def issue_wslab(nb):
            if nb < N_QUARTERED:
                quarters = []
                for q in range(4):
                    t = wqpool.tile([P, 8, N_BLK], FP8, tag="wq")
                    nc.scalar.dma_start(out=t[:], in_=wt_ap[nb][:, 8 * q:8 * q + 8, :])
                    quarters.append(t)
                wslab_tiles[nb] = quarters
            else:
                t = wpool.tile([P, K_SUBS, N_BLK], FP8, tag="w")
                nc.scalar.dma_start(out=t[:], in_=wt_ap[nb])
                wslab_tiles[nb] = t

        def slab_rhs(nb, j):
            if nb < N_QUARTERED:
                jj = j % 4
                return wslab_tiles[nb][j // 4][:, 2 * jj:2 * jj + 2, :]
            return wslab_tiles[nb][:, 2 * j:2 * j + 2, :]

        issue_x(0)
        issue_x(1)
        issue_wslab(0)
        issue_wslab(1)

        ident = singles.tile([P, P], FP8)
        make_identity(nc, ident)

        ws_b = singles.tile([P, N], F32)
        nc.scalar.dma_start(
            out=ws_b[:],
            in_=bass.AP(tensor=ws_d[:].tensor, offset=0, ap=[[0, P], [1, N]]),
        )
        bias_b = singles.tile([P, N], F32)
        nc.scalar.dma_start(
            out=bias_b[:],
            in_=bass.AP(tensor=bias_d[:].tensor, offset=0, ap=[[0, P], [1, N]]),
        )

        xs_tiles = []
        xqt_tiles = []
        prev_inv_inst = None

        def epilogue(mt, nb, pm, phase1):
            # out = bf16(psum * xs[m] * ws[n]) + bias[n]; the fused
            # scalar_tensor_tensor keeps a single rounding to bf16.
            sb1 = opool.tile([P, N_BLK], BF16, tag="sb1")
            nc.vector.scalar_tensor_tensor(
                out=sb1[:], in0=pm[:], scalar=xs_tiles[mt][:],
                in1=ws_b[:, nb * N_BLK:(nb + 1) * N_BLK],
                op0=mybir.AluOpType.mult, op1=mybir.AluOpType.mult,
            )
            sb2 = opool.tile([P, N_BLK], BF16, tag="sb2")
            eng = nc.gpsimd if phase1 else nc.vector
            eng.tensor_add(sb2[:], sb1[:], bias_b[:, nb * N_BLK:(nb + 1) * N_BLK])
            nc.sync.dma_start(
                out=out_ap[mt * P:(mt + 1) * P, nb * N_BLK:(nb + 1) * N_BLK],
                in_=sb2[:],
            )

        def gemm_block(mt, nb, phase1=False):
            pm = psum_mm.tile([P, N_BLK], F32, tag="pm")
            for j in range(K_SUPERS):
                g, jj = divmod(j, 4)
                nc.tensor.matmul(
                    out=pm[:],
                    lhsT=xqt_tiles[mt][g][:, 2 * jj:2 * jj + 2, :],
                    rhs=slab_rhs(nb, j),
                    start=(j == 0), stop=(j == K_SUPERS - 1),
                    perf_mode=mybir.MatmulPerfMode.DoubleRow,
                )
            epilogue(mt, nb, pm, phase1)

        # ---- phase 1: per 128-row tile: quantize, transpose, and two
        # N-blocks of GEMM (keeps the PE saturated while later tiles
        # quantize).
        for mt in range(M_TILES):
            if mt + 2 < M_TILES:
                issue_x(mt + 2)
            if mt <= 1:
                issue_wslab(mt + 2)

            xt = x_tiles[mt]
            amax_a = stats.tile([P, 1], F32, tag="amax_a")
            reduce_inst = nc.vector.tensor_reduce(
                out=amax_a[:], in_=xt[:, 0:KH],
                axis=mybir.AxisListType.X, op=mybir.AluOpType.max,
                apply_absolute_value=True,
            )
            # keep the DVE from scheduling this tile's reduce ahead of the
            # previous tile's tiny scale chain (which gates ACT quant)
            if prev_inv_inst is not None:
                tile.add_dep_helper(reduce_inst.ins, prev_inv_inst.ins, sync=False,
                                    reason="stats chain before next reduce")
            amax_b = stats.tile([P, 1], F32, tag="amax_b")
            nc.vector.tensor_reduce(
                out=amax_b[:], in_=xt[:, KH:K],
                axis=mybir.AxisListType.X, op=mybir.AluOpType.max,
                apply_absolute_value=True,
            )
            with tc.high_priority():
                amax = stats.tile([P, 1], F32, tag="amax")
                nc.vector.tensor_max(amax[:], amax_a[:], amax_b[:])
                # xs = max(amax, eps) * (1/224); quant scale is exactly 1/xs
                xs = xspool.tile([P, 1], F32, tag="xs")
                nc.vector.tensor_scalar(
                    out=xs[:], in0=amax[:],
                    scalar1=1e-10, scalar2=1.0 / 224.0,
                    op0=mybir.AluOpType.max, op1=mybir.AluOpType.mult,
                )
                xs_tiles.append(xs)
                inv = stats.tile([P, 1], F32, tag="inv")
                prev_inv_inst = nc.vector.reciprocal(out=inv[:], in_=xs[:])

            # tile 0 quantizes in halves so its transposes start earlier;
            # later tiles overlap fully and use one ACT op
            if mt == 0:
                xq_parts = []
                for h in range(2):
                    xq_h = xqpool.tile([P, KH], FP8, tag=f"xq{h}")
                    nc.scalar.activation(
                        out=xq_h[:], in_=xt[:, h * KH:(h + 1) * KH],
                        func=mybir.ActivationFunctionType.Copy, scale=inv[:],
                    )
                    xq_parts.append(xq_h)

                def xq_chunk(ks):
                    return xq_parts[ks // 16][:, (ks % 16) * P:(ks % 16 + 1) * P]
            else:
                xq = xqpool.tile([P, K], FP8, tag="xq")
                nc.scalar.activation(
                    out=xq[:], in_=xt[:],
                    func=mybir.ActivationFunctionType.Copy, scale=inv[:],
                )

                def xq_chunk(ks):
                    return xq[:, ks * P:(ks + 1) * P]

            # transpose x_q into [K, M] layout via PE transpose matmuls;
            # evict each 16-ksub half of PSUM to SBUF as one ACT copy
            xqt_groups = []
            for half in range(2):
                if USE_IS_TRANSPOSE:
                    # fp8 transpose mode writes elements on a 2-byte step
                    ptr = psum_tr.tile([P, 16, 2 * P], FP8, tag="ptr")
                    ptr_view = ptr[:, :, 0:2 * P:2]
                else:
                    ptr = psum_tr.tile([P, 16, P], F32, tag="ptr")
                    ptr_view = ptr[:]
                for i in range(16):
                    nc.tensor.matmul(
                        out=ptr_view[:, i, :],
                        lhsT=xq_chunk(half * 16 + i),
                        rhs=ident[:],
                        start=True, stop=True,
                        is_transpose=USE_IS_TRANSPOSE,
                    )
                for g in range(2):
                    xqt_g = xqtpool.tile([P, 8, P], FP8, tag=f"xqt{2 * half + g}")
                    xqt_groups.append(xqt_g)
                    nc.scalar.copy(out=xqt_g[:], in_=ptr_view[:, 8 * g:8 * g + 8, :])
            xqt_tiles.append(xqt_groups)

            for nb in range(NB_PHASE1):
                gemm_block(mt, nb, phase1=True)

        # ---- phase 2: pure fp8 DoubleRow GEMM over the remaining N-blocks
        for nb in range(NB_PHASE1, N_BLKS):
            if nb + 2 < N_BLKS:
                issue_wslab(nb + 2)
            for mt in range(M_TILES):
                gemm_block(mt, nb)

    nc.compile()
    return nc


def _get_program():
    if "nc" not in _PROGRAM_CACHE:
        _PROGRAM_CACHE["nc"] = _build_program()
    return _PROGRAM_CACHE["nc"]


def _run_sharded(x, weight, weight_scales, bias, trace=False):
    x = np.asarray(x).astype(ml_dtypes.bfloat16, copy=False)
    weight = np.asarray(weight, dtype=np.float32)
    weight_scales = np.asarray(weight_scales, dtype=np.float32)
    bias = np.asarray(bias, dtype=np.float32)

    # host-side sharding / layout only:
    # wt[nb, p, ksub, n] = weight[nb*512 + n, ksub*128 + p], re-encoded to
    # fp8 e4m3 (lossless: the reference weights are fp8-round-tripped values)
    wt = np.ascontiguousarray(
        weight.T.reshape(K_SUBS, P, N_BLKS, N_BLK).transpose(2, 1, 0, 3)
    ).astype(ml_dtypes.float8_e4m3)
    in_maps = []
    for c in range(NCORES):
        in_maps.append({
            "x": np.ascontiguousarray(x[c * M_SHARD:(c + 1) * M_SHARD]),
            "wt": wt,
            "ws": weight_scales,
            "bias": bias,
        })

    nc = _get_program()
    res = run_bass_kernel_spmd(nc, in_maps, core_ids=list(range(NCORES)), trace=trace)
    out = np.concatenate([res.results[c]["out"] for c in range(NCORES)], axis=0)
    return out, res.exec_time_ns


def kernel(x, weight, weight_scales, bias):
    out, _ = _run_sharded(x, weight, weight_scales, bias,
                          trace=bool(os.environ.get("KERNEL_TRACE")))
    return out


# revision 18
# speedup vs baseline: 1.0168x; 1.0168x over previous
"""Fp8 per-token/per-channel quantized linear for Trainium2, 8 NeuronCores.

Computation (matches the jax reference):
    amax[m]  = max_k |x[m, k]|                       (x is bf16)
    xs[m]    = max(amax, 1e-10) / 448
    x_q      = e4m3fn_round(x / xs)                  (values up to +-448)
    out      = bf16((x_q @ W^T) * xs * w_scales) + bf16(bias)

Mapping to TRN2 hardware:
  * TRN's fp8 E4M3 saturates at +-240 (256..448 are Inf/NaN), so we quantize
    at HALF scale: x_q' = e4m3_round(x * (224/amax)) == x_q / 2 exactly (the
    fp8 grid is self-similar under powers of two), and fold the factor 2 into
    the output scale: out = psum * (amax/224) * w_scales.  The reference
    weights are already exactly fp8-representable, so casting them is lossless.
  * Sharding: row-parallel over M (8 cores x 1024 rows).  Each core quantizes
    only its own rows, and streams the full weight, transposed on host to
    [K, N] tile layout and losslessly re-encoded to fp8.
  * x_q is transposed on-chip into [K, M] layout with PE transpose matmuls
    (contraction must sit on partitions for both matmul operands).
  * Main GEMM runs in fp8 with perf_mode=DoubleRow (k=256 per matmul).

Schedule (v4): the kernel is PE-bound (DoubleRow GEMM ~221us + transposes
~25us).  The per-tile producer chain is DVE amax (f32 reduce, no fast DVE
mode exists: ~4.6us) + tiny scale chain, ACT quant copy (~3.7us) and two
ACT psum evicts (~4.1us); DVE additionally runs the fused epilogue
(psum*xs*ws in one scalar_tensor_tensor).  ACT (~7.9us) and DVE (~6.6us)
both stay under the PE's 8.6us per-tile consumption of T(mt) + GEMM(mt,0)
+ GEMM(mt,1), so phase 1 runs PE-bound; phase 2 is pure GEMM.  DMA queue
fairness is round-robin per ~descriptor, so x tiles load as single [128,4096]
DMAs (8KB runs) on the sync ring while weight slabs ride the scalar ring,
the first two split into 4KB-run quarters (finer deps + fairness).
"""

import os
import numpy as np
import ml_dtypes
from contextlib import ExitStack

import concourse.bass as bass
import concourse.bacc as bacc
import concourse.tile as tile
from concourse import mybir
from concourse.bass_utils import run_bass_kernel_spmd
from concourse.masks import make_identity

P = 128
M, K, N = 8192, 4096, 4096
NCORES = 8
M_SHARD = M // NCORES          # 1024 rows of x per core
M_TILES = M_SHARD // P         # 8
K_SUBS = K // P                # 32
K_SUPERS = K // (2 * P)        # 16 (DoubleRow consumes 256 rows of K)
KH = K // 2                    # 2048, half-tile for split reduces
N_BLK = 512
N_BLKS = N // N_BLK            # 8
NB_PHASE1 = 2                  # GEMM N-blocks interleaved into the quant loop

FP8 = mybir.dt.float8e4
F32 = mybir.dt.float32
BF16 = mybir.dt.bfloat16

USE_IS_TRANSPOSE = True

_PROGRAM_CACHE = {}


def _build_program():
    nc = bacc.Bacc(None, target_bir_lowering=False)

    x_d = nc.declare_dram_parameter("x", [M_SHARD, K], BF16, isOutput=False)
    # host layout: wt[nb, p, ksub, n] = weight[nb*512 + n, ksub*128 + p],
    # losslessly re-encoded to fp8 (reference weights are fp8-round-tripped)
    wt_d = nc.declare_dram_parameter("wt", [N_BLKS, P, K_SUBS, N_BLK], FP8, isOutput=False)
    ws_d = nc.declare_dram_parameter("ws", [N], F32, isOutput=False)
    bias_d = nc.declare_dram_parameter("bias", [N], F32, isOutput=False)
    out_d = nc.declare_dram_parameter("out", [M_SHARD, N], BF16, isOutput=True)

    x_ap = x_d[:]
    wt_ap = wt_d[:]
    out_ap = out_d[:]

    with tile.TileContext(nc) as tc, ExitStack() as ctx:
        singles = ctx.enter_context(tc.tile_pool(name="singles", bufs=1))
        xpool = ctx.enter_context(tc.tile_pool(name="xpool", bufs=3))
        xqpool = ctx.enter_context(tc.tile_pool(name="xqpool", bufs=2))
        stats = ctx.enter_context(tc.tile_pool(name="stats", bufs=4))
        xspool = ctx.enter_context(tc.tile_pool(name="xspool", bufs=M_TILES))
        xqtpool = ctx.enter_context(tc.tile_pool(name="xqtpool", bufs=M_TILES))
        wqpool = ctx.enter_context(tc.tile_pool(name="wqpool", bufs=12))
        opool = ctx.enter_context(tc.tile_pool(name="opool", bufs=4))
        psum_tr = ctx.enter_context(tc.tile_pool(name="psum_tr", bufs=2, space="PSUM"))
        psum_mm = ctx.enter_context(tc.tile_pool(name="psum_mm", bufs=4, space="PSUM"))

        # ---- upfront DMA issue: x tiles 0-1 on the sync ring; weight slabs
        # on the scalar ring (first two quartered); ws/bias broadcasts are
        # HBM-read-light and use the scalar ring's broadcast path.
        x_tiles = [None] * M_TILES

        def issue_x(mt):
            t = xpool.tile([P, K], BF16, tag="xt")
            nc.sync.dma_start(out=t[:], in_=x_ap[mt * P:(mt + 1) * P, :])
            x_tiles[mt] = t

        wslab_tiles = [None] * N_BLKS

        def issue_wslab(nb, engine):
            # every slab loads as 4 quarter-DMAs: 4KB-per-partition runs keep
            # the round-robin DMA queues fair vs the 8KB x rows, and give the
            # GEMM quarter-granular deps on the arriving weights
            quarters = []
            for q in range(4):
                t = wqpool.tile([P, 8, N_BLK], FP8, tag="wq")
                engine.dma_start(out=t[:], in_=wt_ap[nb][:, 8 * q:8 * q + 8, :])
                quarters.append(t)
            wslab_tiles[nb] = quarters

        def slab_rhs(nb, j):
            jj = j % 4
            return wslab_tiles[nb][j // 4][:, 2 * jj:2 * jj + 2, :]

        issue_x(0)
        issue_x(1)
        issue_wslab(0, nc.scalar)
        issue_wslab(1, nc.scalar)

        ident = singles.tile([P, P], FP8)
        make_identity(nc, ident)

        ws_b = singles.tile([P, N], F32)
        nc.scalar.dma_start(
            out=ws_b[:],
            in_=bass.AP(tensor=ws_d[:].tensor, offset=0, ap=[[0, P], [1, N]]),
        )
        bias_b = singles.tile([P, N], F32)
        nc.scalar.dma_start(
            out=bias_b[:],
            in_=bass.AP(tensor=bias_d[:].tensor, offset=0, ap=[[0, P], [1, N]]),
        )

        xs_tiles = []
        xqt_tiles = []
        prev_inv_inst = None

        def epilogue(mt, nb, pm, phase1):
            # out = bf16(psum * xs[m] * ws[n]) + bias[n]; the fused
            # scalar_tensor_tensor keeps a single rounding to bf16.
            sb1 = opool.tile([P, N_BLK], BF16, tag="sb1")
            nc.vector.scalar_tensor_tensor(
                out=sb1[:], in0=pm[:], scalar=xs_tiles[mt][:],
                in1=ws_b[:, nb * N_BLK:(nb + 1) * N_BLK],
                op0=mybir.AluOpType.mult, op1=mybir.AluOpType.mult,
            )
            sb2 = opool.tile([P, N_BLK], BF16, tag="sb2")
            eng = nc.gpsimd if phase1 else nc.vector
            eng.tensor_add(sb2[:], sb1[:], bias_b[:, nb * N_BLK:(nb + 1) * N_BLK])
            nc.sync.dma_start(
                out=out_ap[mt * P:(mt + 1) * P, nb * N_BLK:(nb + 1) * N_BLK],
                in_=sb2[:],
            )

        def gemm_block(mt, nb, phase1=False):
            pm = psum_mm.tile([P, N_BLK], F32, tag="pm")
            for j in range(K_SUPERS):
                g, jj = divmod(j, 4)
                nc.tensor.matmul(
                    out=pm[:],
                    lhsT=xqt_tiles[mt][g][:, 2 * jj:2 * jj + 2, :],
                    rhs=slab_rhs(nb, j),
                    start=(j == 0), stop=(j == K_SUPERS - 1),
                    perf_mode=mybir.MatmulPerfMode.DoubleRow,
                )
            epilogue(mt, nb, pm, phase1)

        # ---- phase 1: per 128-row tile: quantize, transpose, and two
        # N-blocks of GEMM (keeps the PE saturated while later tiles
        # quantize).
        for mt in range(M_TILES):
            if mt + 2 < M_TILES:
                issue_x(mt + 2)
            if mt == 2:
                issue_wslab(2, nc.sync)

            xt = x_tiles[mt]
            amax_a = stats.tile([P, 1], F32, tag="amax_a")
            reduce_a = nc.vector.tensor_reduce(
                out=amax_a[:], in_=xt[:, 0:KH],
                axis=mybir.AxisListType.X, op=mybir.AluOpType.max,
                apply_absolute_value=True,
            )
            amax_b = stats.tile([P, 1], F32, tag="amax_b")
            reduce_b = nc.vector.tensor_reduce(
                out=amax_b[:], in_=xt[:, KH:K],
                axis=mybir.AxisListType.X, op=mybir.AluOpType.max,
                apply_absolute_value=True,
            )
            # order this tile's reduces after the previous tile's scale
            # chain so the 2.3us reduces don't delay the chain that gates
            # ACT quant
            if prev_inv_inst is not None:
                tile.add_dep_helper(reduce_a.ins, prev_inv_inst.ins, sync=False,
                                    reason="stats chain before next reduce")
                tile.add_dep_helper(reduce_b.ins, prev_inv_inst.ins, sync=False,
                                    reason="stats chain before next reduce")
            with tc.high_priority():
                amax = stats.tile([P, 1], F32, tag="amax")
                nc.vector.tensor_max(amax[:], amax_a[:], amax_b[:])
                # xs = max(amax, eps) * (1/224); quant scale is exactly 1/xs
                xs = xspool.tile([P, 1], F32, tag="xs")
                nc.vector.tensor_scalar(
                    out=xs[:], in0=amax[:],
                    scalar1=1e-10, scalar2=1.0 / 224.0,
                    op0=mybir.AluOpType.max, op1=mybir.AluOpType.mult,
                )
                xs_tiles.append(xs)
                inv = stats.tile([P, 1], F32, tag="inv")
                prev_inv_inst = nc.vector.reciprocal(out=inv[:], in_=xs[:])

            # tile 0 quantizes in halves so its transposes start earlier;
            # later tiles overlap fully and use one ACT op
            if mt == 0:
                xq_parts = []
                for h in range(2):
                    xq_h = xqpool.tile([P, KH], FP8, tag=f"xq{h}")
                    nc.scalar.activation(
                        out=xq_h[:], in_=xt[:, h * KH:(h + 1) * KH],
                        func=mybir.ActivationFunctionType.Copy, scale=inv[:],
                    )
                    xq_parts.append(xq_h)

                def xq_chunk(ks):
                    return xq_parts[ks // 16][:, (ks % 16) * P:(ks % 16 + 1) * P]
            else:
                xq = xqpool.tile([P, K], FP8, tag="xq")
                nc.scalar.activation(
                    out=xq[:], in_=xt[:],
                    func=mybir.ActivationFunctionType.Copy, scale=inv[:],
                )

                def xq_chunk(ks):
                    return xq[:, ks * P:(ks + 1) * P]

            # transpose x_q into [K, M] layout via PE transpose matmuls;
            # evict each 16-ksub half of PSUM to SBUF as one ACT copy
            xqt_groups = []
            for half in range(2):
                if USE_IS_TRANSPOSE:
                    # fp8 transpose mode writes elements on a 2-byte step
                    ptr = psum_tr.tile([P, 16, 2 * P], FP8, tag="ptr")
                    ptr_view = ptr[:, :, 0:2 * P:2]
                else:
                    ptr = psum_tr.tile([P, 16, P], F32, tag="ptr")
                    ptr_view = ptr[:]
                for i in range(16):
                    nc.tensor.matmul(
                        out=ptr_view[:, i, :],
                        lhsT=xq_chunk(half * 16 + i),
                        rhs=ident[:],
                        start=True, stop=True,
                        is_transpose=USE_IS_TRANSPOSE,
                    )
                for g in range(2):
                    xqt_g = xqtpool.tile([P, 8, P], FP8, tag=f"xqt{2 * half + g}")
                    xqt_groups.append(xqt_g)
                    nc.scalar.copy(out=xqt_g[:], in_=ptr_view[:, 8 * g:8 * g + 8, :])
            xqt_tiles.append(xqt_groups)

            for nb in range(NB_PHASE1):
                gemm_block(mt, nb, phase1=True)

        # ---- phase 2: pure fp8 DoubleRow GEMM over the remaining N-blocks
        for nb in range(NB_PHASE1, N_BLKS):
            if nb + 1 < N_BLKS:
                issue_wslab(nb + 1, nc.scalar)
            for mt in range(M_TILES):
                gemm_block(mt, nb)

    nc.compile()
    return nc


def _get_program():
    if "nc" not in _PROGRAM_CACHE:
        _PROGRAM_CACHE["nc"] = _build_program()
    return _PROGRAM_CACHE["nc"]


def _run_sharded(x, weight, weight_scales, bias, trace=False):
    x = np.asarray(x).astype(ml_dtypes.bfloat16, copy=False)
    weight = np.asarray(weight, dtype=np.float32)
    weight_scales = np.asarray(weight_scales, dtype=np.float32)
    bias = np.asarray(bias, dtype=np.float32)

    # host-side sharding / layout only:
    # wt[nb, p, ksub, n] = weight[nb*512 + n, ksub*128 + p], re-encoded to
    # fp8 e4m3 (lossless: the reference weights are fp8-round-tripped values)
    wt = np.ascontiguousarray(
        weight.T.reshape(K_SUBS, P, N_BLKS, N_BLK).transpose(2, 1, 0, 3)
    ).astype(ml_dtypes.float8_e4m3)
    in_maps = []
    for c in range(NCORES):
        in_maps.append({
            "x": np.ascontiguousarray(x[c * M_SHARD:(c + 1) * M_SHARD]),
            "wt": wt,
            "ws": weight_scales,
            "bias": bias,
        })

    nc = _get_program()
    res = run_bass_kernel_spmd(nc, in_maps, core_ids=list(range(NCORES)), trace=trace)
    out = np.concatenate([res.results[c]["out"] for c in range(NCORES)], axis=0)
    return out, res.exec_time_ns


def kernel(x, weight, weight_scales, bias):
    out, _ = _run_sharded(x, weight, weight_scales, bias,
                          trace=bool(os.environ.get("KERNEL_TRACE")))
    return out


# revision 21
# speedup vs baseline: 1.0492x; 1.0319x over previous
"""Fp8 per-token/per-channel quantized linear for Trainium2, 8 NeuronCores.

Computation (matches the jax reference):
    amax[m]  = max_k |x[m, k]|                       (x is bf16)
    xs[m]    = max(amax, 1e-10) / 448
    x_q      = e4m3fn_round(x / xs)                  (values up to +-448)
    out      = bf16((x_q @ W^T) * xs * w_scales) + bf16(bias)

Mapping to TRN2 hardware:
  * TRN's fp8 E4M3 saturates at +-240 (256..448 are Inf/NaN), so we quantize
    at HALF scale: x_q' = e4m3_round(x * (224/amax)) == x_q / 2 exactly (the
    fp8 grid is self-similar under powers of two), and fold the factor 2 into
    the output scale: out = psum * (amax/224) * w_scales.  The reference
    weights are already exactly fp8-representable, so casting them is lossless.
  * Sharding: row-parallel over M (8 cores x 1024 rows).  Each core quantizes
    only its own rows, and streams the full weight, transposed on host to
    [K, N] tile layout and losslessly re-encoded to fp8.
  * x_q is transposed on-chip into [K, M] layout with PE transpose matmuls
    (contraction must sit on partitions for both matmul operands).
  * Main GEMM runs in fp8 with perf_mode=DoubleRow (k=256 per matmul).

Schedule (v4): the kernel is PE-bound (DoubleRow GEMM ~221us + transposes
~25us).  The per-tile producer chain is DVE amax (f32 reduce, no fast DVE
mode exists: ~4.6us) + tiny scale chain, ACT quant copy (~3.7us) and two
ACT psum evicts (~4.1us); DVE additionally runs the fused epilogue
(psum*xs*ws in one scalar_tensor_tensor).  ACT (~7.9us) and DVE (~6.6us)
both stay under the PE's 8.6us per-tile consumption of T(mt) + GEMM(mt,0)
+ GEMM(mt,1), so phase 1 runs PE-bound; phase 2 is pure GEMM.  DMA queue
fairness is round-robin per ~descriptor, so x tiles load as single [128,4096]
DMAs (8KB runs) on the sync ring while weight slabs ride the scalar ring,
the first two split into 4KB-run quarters (finer deps + fairness).
"""

import os
import numpy as np
import ml_dtypes
from contextlib import ExitStack

import concourse.bass as bass
import concourse.bacc as bacc
import concourse.tile as tile
from concourse import mybir
from concourse.bass_utils import run_bass_kernel_spmd
from concourse.masks import make_identity

P = 128
M, K, N = 8192, 4096, 4096
NCORES = 8
M_SHARD = M // NCORES          # 1024 rows of x per core
M_TILES = M_SHARD // P         # 8
K_SUBS = K // P                # 32
K_SUPERS = K // (2 * P)        # 16 (DoubleRow consumes 256 rows of K)
KH = K // 2                    # 2048, half-tile for split reduces
N_BLK = 512
N_BLKS = N // N_BLK            # 8
NB_PHASE1 = 2                  # GEMM N-blocks interleaved into the quant loop

FP8 = mybir.dt.float8e4
F32 = mybir.dt.float32
BF16 = mybir.dt.bfloat16

USE_IS_TRANSPOSE = True

_PROGRAM_CACHE = {}


def _build_program():
    nc = bacc.Bacc(None, target_bir_lowering=False)

    x_d = nc.declare_dram_parameter("x", [M_SHARD, K], BF16, isOutput=False)
    # host layout: wt[nb, p, ksub, n] = weight[nb*512 + n, ksub*128 + p],
    # losslessly re-encoded to fp8 (reference weights are fp8-round-tripped)
    wt_d = nc.declare_dram_parameter("wt", [N_BLKS, P, K_SUBS, N_BLK], FP8, isOutput=False)
    ws_d = nc.declare_dram_parameter("ws", [N], F32, isOutput=False)
    bias_d = nc.declare_dram_parameter("bias", [N], F32, isOutput=False)
    out_d = nc.declare_dram_parameter("out", [M_SHARD, N], BF16, isOutput=True)

    x_ap = x_d[:]
    wt_ap = wt_d[:]
    out_ap = out_d[:]

    with tile.TileContext(nc) as tc, ExitStack() as ctx:
        singles = ctx.enter_context(tc.tile_pool(name="singles", bufs=1))
        xpool = ctx.enter_context(tc.tile_pool(name="xpool", bufs=3))
        xqpool = ctx.enter_context(tc.tile_pool(name="xqpool", bufs=2))
        stats = ctx.enter_context(tc.tile_pool(name="stats", bufs=4))
        xspool = ctx.enter_context(tc.tile_pool(name="xspool", bufs=M_TILES))
        xqtpool = ctx.enter_context(tc.tile_pool(name="xqtpool", bufs=M_TILES))
        wqpool = ctx.enter_context(tc.tile_pool(name="wqpool", bufs=12))
        opool = ctx.enter_context(tc.tile_pool(name="opool", bufs=4))
        psum_tr = ctx.enter_context(tc.tile_pool(name="psum_tr", bufs=2, space="PSUM"))
        psum_mm = ctx.enter_context(tc.tile_pool(name="psum_mm", bufs=4, space="PSUM"))

        # ---- upfront DMA issue: x tiles 0-1 on the sync ring; weight slabs
        # on the scalar ring (first two quartered); ws/bias broadcasts are
        # HBM-read-light and use the scalar ring's broadcast path.
        x_tiles = [None] * M_TILES

        def issue_x(mt):
            t = xpool.tile([P, K], BF16, tag="xt")
            nc.sync.dma_start(out=t[:], in_=x_ap[mt * P:(mt + 1) * P, :])
            x_tiles[mt] = t

        wslab_tiles = [None] * N_BLKS

        def issue_wslab(nb, engine):
            # every slab loads as 4 quarter-DMAs: 4KB-per-partition runs keep
            # the round-robin DMA queues fair vs the 8KB x rows, and give the
            # GEMM quarter-granular deps on the arriving weights
            quarters = []
            for q in range(4):
                t = wqpool.tile([P, 8, N_BLK], FP8, tag="wq")
                engine.dma_start(out=t[:], in_=wt_ap[nb][:, 8 * q:8 * q + 8, :])
                quarters.append(t)
            wslab_tiles[nb] = quarters

        def slab_rhs(nb, j):
            jj = j % 4
            return wslab_tiles[nb][j // 4][:, 2 * jj:2 * jj + 2, :]

        # all bulk DMAs ride the sync ring: doorbells block when the DGE ring
        # fills, and the scalar ring's doorbells would stall ACT compute
        # queued behind them
        issue_x(0)
        issue_x(1)
        issue_wslab(0, nc.sync)
        issue_wslab(1, nc.sync)

        ident = singles.tile([P, P], FP8)
        make_identity(nc, ident)

        ws_b = singles.tile([P, N], F32)
        nc.scalar.dma_start(
            out=ws_b[:],
            in_=bass.AP(tensor=ws_d[:].tensor, offset=0, ap=[[0, P], [1, N]]),
        )
        bias_b = singles.tile([P, N], F32)
        nc.scalar.dma_start(
            out=bias_b[:],
            in_=bass.AP(tensor=bias_d[:].tensor, offset=0, ap=[[0, P], [1, N]]),
        )

        xs_tiles = []
        xqt_tiles = []
        prev_inv_inst = None

        def epilogue(mt, nb, pm, phase1):
            # out = bf16(psum * xs[m] * ws[n]) + bias[n]; the fused
            # scalar_tensor_tensor keeps a single rounding to bf16.
            sb1 = opool.tile([P, N_BLK], BF16, tag="sb1")
            nc.vector.scalar_tensor_tensor(
                out=sb1[:], in0=pm[:], scalar=xs_tiles[mt][:],
                in1=ws_b[:, nb * N_BLK:(nb + 1) * N_BLK],
                op0=mybir.AluOpType.mult, op1=mybir.AluOpType.mult,
            )
            sb2 = opool.tile([P, N_BLK], BF16, tag="sb2")
            eng = nc.gpsimd if phase1 else nc.vector
            eng.tensor_add(sb2[:], sb1[:], bias_b[:, nb * N_BLK:(nb + 1) * N_BLK])
            nc.sync.dma_start(
                out=out_ap[mt * P:(mt + 1) * P, nb * N_BLK:(nb + 1) * N_BLK],
                in_=sb2[:],
            )

        def gemm_block(mt, nb, phase1=False):
            pm = psum_mm.tile([P, N_BLK], F32, tag="pm")
            for j in range(K_SUPERS):
                g, jj = divmod(j, 4)
                nc.tensor.matmul(
                    out=pm[:],
                    lhsT=xqt_tiles[mt][g][:, 2 * jj:2 * jj + 2, :],
                    rhs=slab_rhs(nb, j),
                    start=(j == 0), stop=(j == K_SUPERS - 1),
                    perf_mode=mybir.MatmulPerfMode.DoubleRow,
                )
            epilogue(mt, nb, pm, phase1)

        # ---- phase 1: per 128-row tile: quantize, transpose, and two
        # N-blocks of GEMM (keeps the PE saturated while later tiles
        # quantize).
        for mt in range(M_TILES):
            if mt + 2 < M_TILES:
                issue_x(mt + 2)
            if mt == 2:
                issue_wslab(2, nc.sync)

            xt = x_tiles[mt]
            amax_a = stats.tile([P, 1], F32, tag="amax_a")
            reduce_a = nc.vector.tensor_reduce(
                out=amax_a[:], in_=xt[:, 0:KH],
                axis=mybir.AxisListType.X, op=mybir.AluOpType.max,
                apply_absolute_value=True,
            )
            amax_b = stats.tile([P, 1], F32, tag="amax_b")
            reduce_b = nc.vector.tensor_reduce(
                out=amax_b[:], in_=xt[:, KH:K],
                axis=mybir.AxisListType.X, op=mybir.AluOpType.max,
                apply_absolute_value=True,
            )
            # order this tile's reduces after the previous tile's scale
            # chain so the 2.3us reduces don't delay the chain that gates
            # ACT quant
            if prev_inv_inst is not None:
                tile.add_dep_helper(reduce_a.ins, prev_inv_inst.ins, sync=False,
                                    reason="stats chain before next reduce")
                tile.add_dep_helper(reduce_b.ins, prev_inv_inst.ins, sync=False,
                                    reason="stats chain before next reduce")
            with tc.high_priority():
                amax = stats.tile([P, 1], F32, tag="amax")
                nc.vector.tensor_max(amax[:], amax_a[:], amax_b[:])
                # xs = max(amax, eps) * (1/224); quant scale is exactly 1/xs
                xs = xspool.tile([P, 1], F32, tag="xs")
                nc.vector.tensor_scalar(
                    out=xs[:], in0=amax[:],
                    scalar1=1e-10, scalar2=1.0 / 224.0,
                    op0=mybir.AluOpType.max, op1=mybir.AluOpType.mult,
                )
                xs_tiles.append(xs)
                inv = stats.tile([P, 1], F32, tag="inv")
                prev_inv_inst = nc.vector.reciprocal(out=inv[:], in_=xs[:])

            # tile 0 quantizes in halves so its transposes start earlier;
            # later tiles overlap fully and use one ACT op
            if mt == 0:
                xq_parts = []
                for h in range(2):
                    xq_h = xqpool.tile([P, KH], FP8, tag=f"xq{h}")
                    nc.scalar.activation(
                        out=xq_h[:], in_=xt[:, h * KH:(h + 1) * KH],
                        func=mybir.ActivationFunctionType.Copy, scale=inv[:],
                    )
                    xq_parts.append(xq_h)

                def xq_chunk(ks):
                    return xq_parts[ks // 16][:, (ks % 16) * P:(ks % 16 + 1) * P]
            else:
                xq = xqpool.tile([P, K], FP8, tag="xq")
                nc.scalar.activation(
                    out=xq[:], in_=xt[:],
                    func=mybir.ActivationFunctionType.Copy, scale=inv[:],
                )

                def xq_chunk(ks):
                    return xq[:, ks * P:(ks + 1) * P]

            # transpose x_q into [K, M] layout via PE transpose matmuls;
            # evict each 8-ksub group right after its matmuls so the GEMM's
            # j=0..3 can start as soon as the first group lands in SBUF
            xqt_groups = []
            for half in range(2):
                if USE_IS_TRANSPOSE:
                    # fp8 transpose mode writes elements on a 2-byte step
                    ptr = psum_tr.tile([P, 16, 2 * P], FP8, tag="ptr")
                    ptr_view = ptr[:, :, 0:2 * P:2]
                else:
                    ptr = psum_tr.tile([P, 16, P], F32, tag="ptr")
                    ptr_view = ptr[:]
                for g in range(2):
                    for i in range(8):
                        nc.tensor.matmul(
                            out=ptr_view[:, 8 * g + i, :],
                            lhsT=xq_chunk(half * 16 + 8 * g + i),
                            rhs=ident[:],
                            start=True, stop=True,
                            is_transpose=USE_IS_TRANSPOSE,
                        )
                    xqt_g = xqtpool.tile([P, 8, P], FP8, tag=f"xqt{2 * half + g}")
                    xqt_groups.append(xqt_g)
                    nc.scalar.copy(out=xqt_g[:], in_=ptr_view[:, 8 * g:8 * g + 8, :])
            xqt_tiles.append(xqt_groups)

            for nb in range(NB_PHASE1):
                gemm_block(mt, nb, phase1=True)

        # ---- phase 2: pure fp8 DoubleRow GEMM over the remaining N-blocks
        for nb in range(NB_PHASE1, N_BLKS):
            if nb + 1 < N_BLKS:
                issue_wslab(nb + 1, nc.sync)
            for mt in range(M_TILES):
                gemm_block(mt, nb)

    nc.compile()
    return nc


def _get_program():
    if "nc" not in _PROGRAM_CACHE:
        _PROGRAM_CACHE["nc"] = _build_program()
    return _PROGRAM_CACHE["nc"]


def _run_sharded(x, weight, weight_scales, bias, trace=False):
    x = np.asarray(x).astype(ml_dtypes.bfloat16, copy=False)
    weight = np.asarray(weight, dtype=np.float32)
    weight_scales = np.asarray(weight_scales, dtype=np.float32)
    bias = np.asarray(bias, dtype=np.float32)

    # host-side sharding / layout only:
    # wt[nb, p, ksub, n] = weight[nb*512 + n, ksub*128 + p], re-encoded to
    # fp8 e4m3 (lossless: the reference weights are fp8-round-tripped values)
    wt = np.ascontiguousarray(
        weight.T.reshape(K_SUBS, P, N_BLKS, N_BLK).transpose(2, 1, 0, 3)
    ).astype(ml_dtypes.float8_e4m3)
    in_maps = []
    for c in range(NCORES):
        in_maps.append({
            "x": np.ascontiguousarray(x[c * M_SHARD:(c + 1) * M_SHARD]),
            "wt": wt,
            "ws": weight_scales,
            "bias": bias,
        })

    nc = _get_program()
    res = run_bass_kernel_spmd(nc, in_maps, core_ids=list(range(NCORES)), trace=trace)
    out = np.concatenate([res.results[c]["out"] for c in range(NCORES)], axis=0)
    return out, res.exec_time_ns


def kernel(x, weight, weight_scales, bias):
    out, _ = _run_sharded(x, weight, weight_scales, bias,
                          trace=bool(os.environ.get("KERNEL_TRACE")))
    return out


# revision 22
# speedup vs baseline: 1.0587x; 1.0091x over previous
"""Fp8 per-token/per-channel quantized linear for Trainium2, 8 NeuronCores.

Computation (matches the jax reference):
    amax[m]  = max_k |x[m, k]|                       (x is bf16)
    xs[m]    = max(amax, 1e-10) / 448
    x_q      = e4m3fn_round(x / xs)                  (values up to +-448)
    out      = bf16((x_q @ W^T) * xs * w_scales) + bf16(bias)

Mapping to TRN2 hardware:
  * TRN's fp8 E4M3 saturates at +-240 (256..448 are Inf/NaN), so we quantize
    at HALF scale: x_q' = e4m3_round(x * (224/amax)) == x_q / 2 exactly (the
    fp8 grid is self-similar under powers of two), and fold the factor 2 into
    the output scale: out = psum * (amax/224) * w_scales.  The reference
    weights are already exactly fp8-representable, so casting them is lossless.
  * Sharding: row-parallel over M (8 cores x 1024 rows).  Each core quantizes
    only its own rows, and streams the full weight, transposed on host to
    [K, N] tile layout and losslessly re-encoded to fp8.
  * x_q is transposed on-chip into [K, M] layout with PE transpose matmuls
    (contraction must sit on partitions for both matmul operands).
  * Main GEMM runs in fp8 with perf_mode=DoubleRow (k=256 per matmul).

Schedule (v4): the kernel is PE-bound (DoubleRow GEMM ~221us + transposes
~25us).  The per-tile producer chain is DVE amax (f32 reduce, no fast DVE
mode exists: ~4.6us) + tiny scale chain, ACT quant copy (~3.7us) and two
ACT psum evicts (~4.1us); DVE additionally runs the fused epilogue
(psum*xs*ws in one scalar_tensor_tensor).  ACT (~7.9us) and DVE (~6.6us)
both stay under the PE's 8.6us per-tile consumption of T(mt) + GEMM(mt,0)
+ GEMM(mt,1), so phase 1 runs PE-bound; phase 2 is pure GEMM.  DMA queue
fairness is round-robin per ~descriptor, so x tiles load as single [128,4096]
DMAs (8KB runs) on the sync ring while weight slabs ride the scalar ring,
the first two split into 4KB-run quarters (finer deps + fairness).
"""

import os
import numpy as np
import ml_dtypes
from contextlib import ExitStack

import concourse.bass as bass
import concourse.bacc as bacc
import concourse.tile as tile
from concourse import mybir
from concourse.bass_utils import run_bass_kernel_spmd
from concourse.masks import make_identity

P = 128
M, K, N = 8192, 4096, 4096
NCORES = 8
M_SHARD = M // NCORES          # 1024 rows of x per core
M_TILES = M_SHARD // P         # 8
K_SUBS = K // P                # 32
K_SUPERS = K // (2 * P)        # 16 (DoubleRow consumes 256 rows of K)
KH = K // 2                    # 2048, half-tile for split reduces
N_BLK = 512
N_BLKS = N // N_BLK            # 8
NB_PHASE1 = 2                  # GEMM N-blocks interleaved into the quant loop

FP8 = mybir.dt.float8e4
F32 = mybir.dt.float32
BF16 = mybir.dt.bfloat16

USE_IS_TRANSPOSE = True

_PROGRAM_CACHE = {}


def _build_program():
    nc = bacc.Bacc(None, target_bir_lowering=False)

    x_d = nc.declare_dram_parameter("x", [M_SHARD, K], BF16, isOutput=False)
    # host layout: wt[nb, p, ksub, n] = weight[nb*512 + n, ksub*128 + p],
    # losslessly re-encoded to fp8 (reference weights are fp8-round-tripped)
    wt_d = nc.declare_dram_parameter("wt", [N_BLKS, P, K_SUBS, N_BLK], FP8, isOutput=False)
    ws_d = nc.declare_dram_parameter("ws", [N], F32, isOutput=False)
    bias_d = nc.declare_dram_parameter("bias", [N], F32, isOutput=False)
    out_d = nc.declare_dram_parameter("out", [M_SHARD, N], BF16, isOutput=True)

    x_ap = x_d[:]
    wt_ap = wt_d[:]
    out_ap = out_d[:]

    with tile.TileContext(nc) as tc, ExitStack() as ctx:
        singles = ctx.enter_context(tc.tile_pool(name="singles", bufs=1))
        xpool = ctx.enter_context(tc.tile_pool(name="xpool", bufs=3))
        xqpool = ctx.enter_context(tc.tile_pool(name="xqpool", bufs=2))
        stats = ctx.enter_context(tc.tile_pool(name="stats", bufs=4))
        xspool = ctx.enter_context(tc.tile_pool(name="xspool", bufs=M_TILES))
        xqtpool = ctx.enter_context(tc.tile_pool(name="xqtpool", bufs=M_TILES))
        wqpool = ctx.enter_context(tc.tile_pool(name="wqpool", bufs=12))
        opool = ctx.enter_context(tc.tile_pool(name="opool", bufs=4))
        psum_tr = ctx.enter_context(tc.tile_pool(name="psum_tr", bufs=2, space="PSUM"))
        psum_mm = ctx.enter_context(tc.tile_pool(name="psum_mm", bufs=4, space="PSUM"))

        # ---- upfront DMA issue: x tiles 0-1 on the sync ring; weight slabs
        # on the scalar ring (first two quartered); ws/bias broadcasts are
        # HBM-read-light and use the scalar ring's broadcast path.
        x_tiles = [None] * M_TILES

        def issue_x(mt, split=False):
            t = xpool.tile([P, K], BF16, tag="xt")
            if split:
                # first tile rides both DGE rings so the halves land in
                # parallel right after ring warmup
                nc.sync.dma_start(out=t[:, 0:KH], in_=x_ap[mt * P:(mt + 1) * P, 0:KH])
                nc.scalar.dma_start(out=t[:, KH:K], in_=x_ap[mt * P:(mt + 1) * P, KH:K])
            else:
                nc.sync.dma_start(out=t[:], in_=x_ap[mt * P:(mt + 1) * P, :])
            x_tiles[mt] = t

        wslab_tiles = [None] * N_BLKS

        def issue_wslab(nb, engine):
            # every slab loads as 4 quarter-DMAs: 4KB-per-partition runs keep
            # the round-robin DMA queues fair vs the 8KB x rows, and give the
            # GEMM quarter-granular deps on the arriving weights
            quarters = []
            for q in range(4):
                t = wqpool.tile([P, 8, N_BLK], FP8, tag="wq")
                engine.dma_start(out=t[:], in_=wt_ap[nb][:, 8 * q:8 * q + 8, :])
                quarters.append(t)
            wslab_tiles[nb] = quarters

        def slab_rhs(nb, j):
            jj = j % 4
            return wslab_tiles[nb][j // 4][:, 2 * jj:2 * jj + 2, :]

        # all bulk DMAs ride the sync ring: doorbells block when the DGE ring
        # fills, and the scalar ring's doorbells would stall ACT compute
        # queued behind them
        issue_x(0, split=True)
        issue_x(1)
        issue_wslab(0, nc.sync)
        issue_wslab(1, nc.sync)

        ident = singles.tile([P, P], FP8)
        make_identity(nc, ident)

        ws_b = singles.tile([P, N], F32)
        nc.scalar.dma_start(
            out=ws_b[:],
            in_=bass.AP(tensor=ws_d[:].tensor, offset=0, ap=[[0, P], [1, N]]),
        )
        bias_b = singles.tile([P, N], F32)
        nc.scalar.dma_start(
            out=bias_b[:],
            in_=bass.AP(tensor=bias_d[:].tensor, offset=0, ap=[[0, P], [1, N]]),
        )

        xs_tiles = []
        xqt_tiles = []
        prev_inv_inst = None

        def epilogue(mt, nb, pm, phase1):
            # out = bf16(psum * xs[m] * ws[n]) + bias[n]; the fused
            # scalar_tensor_tensor keeps a single rounding to bf16.
            sb1 = opool.tile([P, N_BLK], BF16, tag="sb1")
            nc.vector.scalar_tensor_tensor(
                out=sb1[:], in0=pm[:], scalar=xs_tiles[mt][:],
                in1=ws_b[:, nb * N_BLK:(nb + 1) * N_BLK],
                op0=mybir.AluOpType.mult, op1=mybir.AluOpType.mult,
            )
            sb2 = opool.tile([P, N_BLK], BF16, tag="sb2")
            eng = nc.gpsimd if phase1 else nc.vector
            eng.tensor_add(sb2[:], sb1[:], bias_b[:, nb * N_BLK:(nb + 1) * N_BLK])
            nc.sync.dma_start(
                out=out_ap[mt * P:(mt + 1) * P, nb * N_BLK:(nb + 1) * N_BLK],
                in_=sb2[:],
            )

        def gemm_block(mt, nb, phase1=False):
            pm = psum_mm.tile([P, N_BLK], F32, tag="pm")
            for j in range(K_SUPERS):
                g, jj = divmod(j, 4)
                nc.tensor.matmul(
                    out=pm[:],
                    lhsT=xqt_tiles[mt][g][:, 2 * jj:2 * jj + 2, :],
                    rhs=slab_rhs(nb, j),
                    start=(j == 0), stop=(j == K_SUPERS - 1),
                    perf_mode=mybir.MatmulPerfMode.DoubleRow,
                )
            epilogue(mt, nb, pm, phase1)

        # ---- phase 1: per 128-row tile: quantize, transpose, and two
        # N-blocks of GEMM (keeps the PE saturated while later tiles
        # quantize).
        for mt in range(M_TILES):
            if mt + 2 < M_TILES:
                issue_x(mt + 2)
            if mt == 2:
                issue_wslab(2, nc.sync)

            xt = x_tiles[mt]
            amax_a = stats.tile([P, 1], F32, tag="amax_a")
            reduce_a = nc.vector.tensor_reduce(
                out=amax_a[:], in_=xt[:, 0:KH],
                axis=mybir.AxisListType.X, op=mybir.AluOpType.max,
                apply_absolute_value=True,
            )
            amax_b = stats.tile([P, 1], F32, tag="amax_b")
            reduce_b = nc.vector.tensor_reduce(
                out=amax_b[:], in_=xt[:, KH:K],
                axis=mybir.AxisListType.X, op=mybir.AluOpType.max,
                apply_absolute_value=True,
            )
            # order this tile's reduces after the previous tile's scale
            # chain so the 2.3us reduces don't delay the chain that gates
            # ACT quant
            if prev_inv_inst is not None:
                tile.add_dep_helper(reduce_a.ins, prev_inv_inst.ins, sync=False,
                                    reason="stats chain before next reduce")
                tile.add_dep_helper(reduce_b.ins, prev_inv_inst.ins, sync=False,
                                    reason="stats chain before next reduce")
            with tc.high_priority():
                amax = stats.tile([P, 1], F32, tag="amax")
                nc.vector.tensor_max(amax[:], amax_a[:], amax_b[:])
                # xs = max(amax, eps) * (1/224); quant scale is exactly 1/xs
                xs = xspool.tile([P, 1], F32, tag="xs")
                nc.vector.tensor_scalar(
                    out=xs[:], in0=amax[:],
                    scalar1=1e-10, scalar2=1.0 / 224.0,
                    op0=mybir.AluOpType.max, op1=mybir.AluOpType.mult,
                )
                xs_tiles.append(xs)
                inv = stats.tile([P, 1], F32, tag="inv")
                prev_inv_inst = nc.vector.reciprocal(out=inv[:], in_=xs[:])

            # quantize in halves: the transposes of half A start while
            # half B is still quantizing on ACT
            xq_parts = []
            for h in range(2):
                xq_h = xqpool.tile([P, KH], FP8, tag=f"xq{h}")
                nc.scalar.activation(
                    out=xq_h[:], in_=xt[:, h * KH:(h + 1) * KH],
                    func=mybir.ActivationFunctionType.Copy, scale=inv[:],
                )
                xq_parts.append(xq_h)

            def xq_chunk(ks):
                return xq_parts[ks // 16][:, (ks % 16) * P:(ks % 16 + 1) * P]

            # transpose x_q into [K, M] layout via PE transpose matmuls;
            # evict each 8-ksub group right after its matmuls so the GEMM's
            # j=0..3 can start as soon as the first group lands in SBUF
            xqt_groups = []
            for half in range(2):
                if USE_IS_TRANSPOSE:
                    # fp8 transpose mode writes elements on a 2-byte step
                    ptr = psum_tr.tile([P, 16, 2 * P], FP8, tag="ptr")
                    ptr_view = ptr[:, :, 0:2 * P:2]
                else:
                    ptr = psum_tr.tile([P, 16, P], F32, tag="ptr")
                    ptr_view = ptr[:]
                for g in range(2):
                    for i in range(8):
                        nc.tensor.matmul(
                            out=ptr_view[:, 8 * g + i, :],
                            lhsT=xq_chunk(half * 16 + 8 * g + i),
                            rhs=ident[:],
                            start=True, stop=True,
                            is_transpose=USE_IS_TRANSPOSE,
                        )
                    xqt_g = xqtpool.tile([P, 8, P], FP8, tag=f"xqt{2 * half + g}")
                    xqt_groups.append(xqt_g)
                    nc.scalar.copy(out=xqt_g[:], in_=ptr_view[:, 8 * g:8 * g + 8, :])
            xqt_tiles.append(xqt_groups)

            if mt >= 1:
                gemm_block(mt - 1, 1, phase1=True)
            gemm_block(mt, 0, phase1=True)

        gemm_block(M_TILES - 1, 1, phase1=True)

        # ---- phase 2: pure fp8 DoubleRow GEMM over the remaining N-blocks
        for nb in range(NB_PHASE1, N_BLKS):
            if nb + 1 < N_BLKS:
                issue_wslab(nb + 1, nc.sync)
            for mt in range(M_TILES):
                gemm_block(mt, nb)

    nc.compile()
    return nc


def _get_program():
    if "nc" not in _PROGRAM_CACHE:
        _PROGRAM_CACHE["nc"] = _build_program()
    return _PROGRAM_CACHE["nc"]


def _run_sharded(x, weight, weight_scales, bias, trace=False):
    x = np.asarray(x).astype(ml_dtypes.bfloat16, copy=False)
    weight = np.asarray(weight, dtype=np.float32)
    weight_scales = np.asarray(weight_scales, dtype=np.float32)
    bias = np.asarray(bias, dtype=np.float32)

    # host-side sharding / layout only:
    # wt[nb, p, ksub, n] = weight[nb*512 + n, ksub*128 + p], re-encoded to
    # fp8 e4m3 (lossless: the reference weights are fp8-round-tripped values)
    wt = np.ascontiguousarray(
        weight.T.reshape(K_SUBS, P, N_BLKS, N_BLK).transpose(2, 1, 0, 3)
    ).astype(ml_dtypes.float8_e4m3)
    in_maps = []
    for c in range(NCORES):
        in_maps.append({
            "x": np.ascontiguousarray(x[c * M_SHARD:(c + 1) * M_SHARD]),
            "wt": wt,
            "ws": weight_scales,
            "bias": bias,
        })

    nc = _get_program()
    res = run_bass_kernel_spmd(nc, in_maps, core_ids=list(range(NCORES)), trace=trace)
    out = np.concatenate([res.results[c]["out"] for c in range(NCORES)], axis=0)
    return out, res.exec_time_ns


def kernel(x, weight, weight_scales, bias):
    out, _ = _run_sharded(x, weight, weight_scales, bias,
                          trace=bool(os.environ.get("KERNEL_TRACE")))
    return out


# revision 23
# speedup vs baseline: 1.0629x; 1.0040x over previous
"""Fp8 per-token/per-channel quantized linear for Trainium2, 8 NeuronCores.

Computation (matches the jax reference):
    amax[m]  = max_k |x[m, k]|                       (x is bf16)
    xs[m]    = max(amax, 1e-10) / 448
    x_q      = e4m3fn_round(x / xs)                  (values up to +-448)
    out      = bf16((x_q @ W^T) * xs * w_scales) + bf16(bias)

Mapping to TRN2 hardware:
  * TRN's fp8 E4M3 saturates at +-240 (256..448 are Inf/NaN), so we quantize
    at HALF scale: x_q' = e4m3_round(x * (224/amax)) == x_q / 2 exactly (the
    fp8 grid is self-similar under powers of two), and fold the factor 2 into
    the output scale: out = psum * (amax/224) * w_scales.  The reference
    weights are already exactly fp8-representable, so casting them is lossless.
  * Sharding: row-parallel over M (8 cores x 1024 rows).  Each core quantizes
    only its own rows, and streams the full weight, transposed on host to
    [K, N] tile layout and losslessly re-encoded to fp8.
  * x_q is transposed on-chip into [K, M] layout with PE transpose matmuls
    (contraction must sit on partitions for both matmul operands).
  * Main GEMM runs in fp8 with perf_mode=DoubleRow (k=256 per matmul).

Schedule (v4): the kernel is PE-bound (DoubleRow GEMM ~221us + transposes
~25us).  The per-tile producer chain is DVE amax (f32 reduce, no fast DVE
mode exists: ~4.6us) + tiny scale chain, ACT quant copy (~3.7us) and two
ACT psum evicts (~4.1us); DVE additionally runs the fused epilogue
(psum*xs*ws in one scalar_tensor_tensor).  ACT (~7.9us) and DVE (~6.6us)
both stay under the PE's 8.6us per-tile consumption of T(mt) + GEMM(mt,0)
+ GEMM(mt,1), so phase 1 runs PE-bound; phase 2 is pure GEMM.  DMA queue
fairness is round-robin per ~descriptor, so x tiles load as single [128,4096]
DMAs (8KB runs) on the sync ring while weight slabs ride the scalar ring,
the first two split into 4KB-run quarters (finer deps + fairness).
"""

import os
import numpy as np
import ml_dtypes
from contextlib import ExitStack

import concourse.bass as bass
import concourse.bacc as bacc
import concourse.tile as tile
from concourse import mybir
from concourse.bass_utils import run_bass_kernel_spmd
from concourse.masks import make_identity

P = 128
M, K, N = 8192, 4096, 4096
NCORES = 8
M_SHARD = M // NCORES          # 1024 rows of x per core
M_TILES = M_SHARD // P         # 8
K_SUBS = K // P                # 32
K_SUPERS = K // (2 * P)        # 16 (DoubleRow consumes 256 rows of K)
KH = K // 2                    # 2048, half-tile for split reduces
N_BLK = 512
N_BLKS = N // N_BLK            # 8
NB_PHASE1 = 2                  # GEMM N-blocks interleaved into the quant loop

FP8 = mybir.dt.float8e4
F32 = mybir.dt.float32
BF16 = mybir.dt.bfloat16

USE_IS_TRANSPOSE = True

_PROGRAM_CACHE = {}


def _build_program():
    nc = bacc.Bacc(None, target_bir_lowering=False)

    x_d = nc.declare_dram_parameter("x", [M_SHARD, K], BF16, isOutput=False)
    # host layout: wt[nb, p, ksub, n] = weight[nb*512 + n, ksub*128 + p],
    # losslessly re-encoded to fp8 (reference weights are fp8-round-tripped)
    wt_d = nc.declare_dram_parameter("wt", [N_BLKS, P, K_SUBS, N_BLK], FP8, isOutput=False)
    ws_d = nc.declare_dram_parameter("ws", [N], F32, isOutput=False)
    bias_d = nc.declare_dram_parameter("bias", [N], F32, isOutput=False)
    out_d = nc.declare_dram_parameter("out", [M_SHARD, N], BF16, isOutput=True)

    x_ap = x_d[:]
    wt_ap = wt_d[:]
    out_ap = out_d[:]

    with tile.TileContext(nc) as tc, ExitStack() as ctx:
        singles = ctx.enter_context(tc.tile_pool(name="singles", bufs=1))
        xpool = ctx.enter_context(tc.tile_pool(name="xpool", bufs=3))
        xqpool = ctx.enter_context(tc.tile_pool(name="xqpool", bufs=2))
        stats = ctx.enter_context(tc.tile_pool(name="stats", bufs=4))
        xspool = ctx.enter_context(tc.tile_pool(name="xspool", bufs=M_TILES))
        xqtpool = ctx.enter_context(tc.tile_pool(name="xqtpool", bufs=M_TILES))
        wqpool = ctx.enter_context(tc.tile_pool(name="wqpool", bufs=12))
        opool = ctx.enter_context(tc.tile_pool(name="opool", bufs=4))
        psum_tr = ctx.enter_context(tc.tile_pool(name="psum_tr", bufs=2, space="PSUM"))
        psum_mm = ctx.enter_context(tc.tile_pool(name="psum_mm", bufs=4, space="PSUM"))

        # ---- upfront DMA issue: x tiles 0-1 on the sync ring; weight slabs
        # on the scalar ring (first two quartered); ws/bias broadcasts are
        # HBM-read-light and use the scalar ring's broadcast path.
        x_tiles = [None] * M_TILES

        def issue_x(mt, split=False):
            t = xpool.tile([P, K], BF16, tag="xt")
            if split:
                # first tile rides both DGE rings so the halves land in
                # parallel right after ring warmup
                nc.sync.dma_start(out=t[:, 0:KH], in_=x_ap[mt * P:(mt + 1) * P, 0:KH])
                nc.scalar.dma_start(out=t[:, KH:K], in_=x_ap[mt * P:(mt + 1) * P, KH:K])
            else:
                nc.sync.dma_start(out=t[:], in_=x_ap[mt * P:(mt + 1) * P, :])
            x_tiles[mt] = t

        wslab_tiles = [None] * N_BLKS

        def issue_wslab(nb, engine):
            # every slab loads as 4 quarter-DMAs: 4KB-per-partition runs keep
            # the round-robin DMA queues fair vs the 8KB x rows, and give the
            # GEMM quarter-granular deps on the arriving weights
            quarters = []
            for q in range(4):
                t = wqpool.tile([P, 8, N_BLK], FP8, tag="wq")
                engine.dma_start(out=t[:], in_=wt_ap[nb][:, 8 * q:8 * q + 8, :])
                quarters.append(t)
            wslab_tiles[nb] = quarters

        def slab_rhs(nb, j):
            jj = j % 4
            return wslab_tiles[nb][j // 4][:, 2 * jj:2 * jj + 2, :]

        # all bulk DMAs ride the sync ring: doorbells block when the DGE ring
        # fills, and the scalar ring's doorbells would stall ACT compute
        # queued behind them
        issue_x(0, split=True)
        issue_x(1)
        issue_wslab(0, nc.sync)

        ident = singles.tile([P, P], FP8)
        make_identity(nc, ident)

        ws_b = singles.tile([P, N], F32)
        nc.scalar.dma_start(
            out=ws_b[:],
            in_=bass.AP(tensor=ws_d[:].tensor, offset=0, ap=[[0, P], [1, N]]),
        )
        bias_b = singles.tile([P, N], F32)
        nc.scalar.dma_start(
            out=bias_b[:],
            in_=bass.AP(tensor=bias_d[:].tensor, offset=0, ap=[[0, P], [1, N]]),
        )
        # slab1 rides the otherwise-idle scalar ring; its doorbells clear the
        # DGE ring just before the first quant needs the ACT queue
        issue_wslab(1, nc.scalar)

        xs_tiles = []
        xqt_tiles = []
        prev_inv_inst = None

        def epilogue(mt, nb, pm, phase1):
            # out = bf16(psum * xs[m] * ws[n]) + bias[n]; the fused
            # scalar_tensor_tensor keeps a single rounding to bf16.
            sb1 = opool.tile([P, N_BLK], BF16, tag="sb1")
            nc.vector.scalar_tensor_tensor(
                out=sb1[:], in0=pm[:], scalar=xs_tiles[mt][:],
                in1=ws_b[:, nb * N_BLK:(nb + 1) * N_BLK],
                op0=mybir.AluOpType.mult, op1=mybir.AluOpType.mult,
            )
            sb2 = opool.tile([P, N_BLK], BF16, tag="sb2")
            eng = nc.gpsimd if phase1 else nc.vector
            eng.tensor_add(sb2[:], sb1[:], bias_b[:, nb * N_BLK:(nb + 1) * N_BLK])
            nc.sync.dma_start(
                out=out_ap[mt * P:(mt + 1) * P, nb * N_BLK:(nb + 1) * N_BLK],
                in_=sb2[:],
            )

        def gemm_block(mt, nb, phase1=False):
            pm = psum_mm.tile([P, N_BLK], F32, tag="pm")
            for j in range(K_SUPERS):
                g, jj = divmod(j, 4)
                nc.tensor.matmul(
                    out=pm[:],
                    lhsT=xqt_tiles[mt][g][:, 2 * jj:2 * jj + 2, :],
                    rhs=slab_rhs(nb, j),
                    start=(j == 0), stop=(j == K_SUPERS - 1),
                    perf_mode=mybir.MatmulPerfMode.DoubleRow,
                )
            epilogue(mt, nb, pm, phase1)

        # ---- phase 1: per 128-row tile: quantize, transpose, and two
        # N-blocks of GEMM (keeps the PE saturated while later tiles
        # quantize).
        for mt in range(M_TILES):
            if mt + 2 < M_TILES:
                issue_x(mt + 2)
            if mt == 2:
                issue_wslab(2, nc.sync)

            xt = x_tiles[mt]
            amax_a = stats.tile([P, 1], F32, tag="amax_a")
            reduce_a = nc.vector.tensor_reduce(
                out=amax_a[:], in_=xt[:, 0:KH],
                axis=mybir.AxisListType.X, op=mybir.AluOpType.max,
                apply_absolute_value=True,
            )
            amax_b = stats.tile([P, 1], F32, tag="amax_b")
            reduce_b = nc.vector.tensor_reduce(
                out=amax_b[:], in_=xt[:, KH:K],
                axis=mybir.AxisListType.X, op=mybir.AluOpType.max,
                apply_absolute_value=True,
            )
            # order this tile's reduces after the previous tile's scale
            # chain so the 2.3us reduces don't delay the chain that gates
            # ACT quant
            if prev_inv_inst is not None:
                tile.add_dep_helper(reduce_a.ins, prev_inv_inst.ins, sync=False,
                                    reason="stats chain before next reduce")
                tile.add_dep_helper(reduce_b.ins, prev_inv_inst.ins, sync=False,
                                    reason="stats chain before next reduce")
            with tc.high_priority():
                amax = stats.tile([P, 1], F32, tag="amax")
                nc.vector.tensor_max(amax[:], amax_a[:], amax_b[:])
                # xs = max(amax, eps) * (1/224); quant scale is exactly 1/xs
                xs = xspool.tile([P, 1], F32, tag="xs")
                nc.vector.tensor_scalar(
                    out=xs[:], in0=amax[:],
                    scalar1=1e-10, scalar2=1.0 / 224.0,
                    op0=mybir.AluOpType.max, op1=mybir.AluOpType.mult,
                )
                xs_tiles.append(xs)
                inv = stats.tile([P, 1], F32, tag="inv")
                prev_inv_inst = nc.vector.reciprocal(out=inv[:], in_=xs[:])

            # quantize in halves: the transposes of half A start while
            # half B is still quantizing on ACT
            xq_parts = []
            for h in range(2):
                xq_h = xqpool.tile([P, KH], FP8, tag=f"xq{h}")
                nc.scalar.activation(
                    out=xq_h[:], in_=xt[:, h * KH:(h + 1) * KH],
                    func=mybir.ActivationFunctionType.Copy, scale=inv[:],
                )
                xq_parts.append(xq_h)

            def xq_chunk(ks):
                return xq_parts[ks // 16][:, (ks % 16) * P:(ks % 16 + 1) * P]

            # transpose x_q into [K, M] layout via PE transpose matmuls;
            # evict each 8-ksub group right after its matmuls so the GEMM's
            # j=0..3 can start as soon as the first group lands in SBUF
            xqt_groups = []
            for half in range(2):
                if USE_IS_TRANSPOSE:
                    # fp8 transpose mode writes elements on a 2-byte step
                    ptr = psum_tr.tile([P, 16, 2 * P], FP8, tag="ptr")
                    ptr_view = ptr[:, :, 0:2 * P:2]
                else:
                    ptr = psum_tr.tile([P, 16, P], F32, tag="ptr")
                    ptr_view = ptr[:]
                for g in range(2):
                    for i in range(8):
                        nc.tensor.matmul(
                            out=ptr_view[:, 8 * g + i, :],
                            lhsT=xq_chunk(half * 16 + 8 * g + i),
                            rhs=ident[:],
                            start=True, stop=True,
                            is_transpose=USE_IS_TRANSPOSE,
                        )
                    xqt_g = xqtpool.tile([P, 8, P], FP8, tag=f"xqt{2 * half + g}")
                    xqt_groups.append(xqt_g)
                    nc.scalar.copy(out=xqt_g[:], in_=ptr_view[:, 8 * g:8 * g + 8, :])
            xqt_tiles.append(xqt_groups)

            if mt >= 1:
                gemm_block(mt - 1, 1, phase1=True)
            gemm_block(mt, 0, phase1=True)

        gemm_block(M_TILES - 1, 1, phase1=True)

        # ---- phase 2: pure fp8 DoubleRow GEMM over the remaining N-blocks
        for nb in range(NB_PHASE1, N_BLKS):
            if nb + 1 < N_BLKS:
                issue_wslab(nb + 1, nc.sync)
            for mt in range(M_TILES):
                gemm_block(mt, nb)

    nc.compile()
    return nc


def _get_program():
    if "nc" not in _PROGRAM_CACHE:
        _PROGRAM_CACHE["nc"] = _build_program()
    return _PROGRAM_CACHE["nc"]


def _run_sharded(x, weight, weight_scales, bias, trace=False):
    x = np.asarray(x).astype(ml_dtypes.bfloat16, copy=False)
    weight = np.asarray(weight, dtype=np.float32)
    weight_scales = np.asarray(weight_scales, dtype=np.float32)
    bias = np.asarray(bias, dtype=np.float32)

    # host-side sharding / layout only:
    # wt[nb, p, ksub, n] = weight[nb*512 + n, ksub*128 + p], re-encoded to
    # fp8 e4m3 (lossless: the reference weights are fp8-round-tripped values)
    wt = np.ascontiguousarray(
        weight.T.reshape(K_SUBS, P, N_BLKS, N_BLK).transpose(2, 1, 0, 3)
    ).astype(ml_dtypes.float8_e4m3)
    in_maps = []
    for c in range(NCORES):
        in_maps.append({
            "x": np.ascontiguousarray(x[c * M_SHARD:(c + 1) * M_SHARD]),
            "wt": wt,
            "ws": weight_scales,
            "bias": bias,
        })

    nc = _get_program()
    res = run_bass_kernel_spmd(nc, in_maps, core_ids=list(range(NCORES)), trace=trace)
    out = np.concatenate([res.results[c]["out"] for c in range(NCORES)], axis=0)
    return out, res.exec_time_ns


def kernel(x, weight, weight_scales, bias):
    out, _ = _run_sharded(x, weight, weight_scales, bias,
                          trace=bool(os.environ.get("KERNEL_TRACE")))
    return out


# revision 25
# speedup vs baseline: 1.0724x; 1.0089x over previous
"""Fp8 per-token/per-channel quantized linear for Trainium2, 8 NeuronCores.

Computation (matches the jax reference):
    amax[m]  = max_k |x[m, k]|                       (x is bf16)
    xs[m]    = max(amax, 1e-10) / 448
    x_q      = e4m3fn_round(x / xs)                  (values up to +-448)
    out      = bf16((x_q @ W^T) * xs * w_scales) + bf16(bias)

Mapping to TRN2 hardware:
  * TRN's fp8 E4M3 saturates at +-240 (256..448 are Inf/NaN), so we quantize
    at HALF scale: x_q' = e4m3_round(x * (224/amax)) == x_q / 2 exactly (the
    fp8 grid is self-similar under powers of two), and fold the factor 2 into
    the output scale: out = psum * (amax/224) * w_scales.  The reference
    weights are already exactly fp8-representable, so casting them is lossless.
  * Sharding: row-parallel over M (8 cores x 1024 rows).  Each core quantizes
    only its own rows, and streams the full weight, transposed on host to
    [K, N] tile layout and losslessly re-encoded to fp8.
  * x_q is transposed on-chip into [K, M] layout with PE transpose matmuls
    (contraction must sit on partitions for both matmul operands).
  * Main GEMM runs in fp8 with perf_mode=DoubleRow (k=256 per matmul).

Schedule (v4): the kernel is PE-bound (DoubleRow GEMM ~221us + transposes
~25us).  The per-tile producer chain is DVE amax (f32 reduce, no fast DVE
mode exists: ~4.6us) + tiny scale chain, ACT quant copy (~3.7us) and two
ACT psum evicts (~4.1us); DVE additionally runs the fused epilogue
(psum*xs*ws in one scalar_tensor_tensor).  ACT (~7.9us) and DVE (~6.6us)
both stay under the PE's 8.6us per-tile consumption of T(mt) + GEMM(mt,0)
+ GEMM(mt,1), so phase 1 runs PE-bound; phase 2 is pure GEMM.  DMA queue
fairness is round-robin per ~descriptor, so x tiles load as single [128,4096]
DMAs (8KB runs) on the sync ring while weight slabs ride the scalar ring,
the first two split into 4KB-run quarters (finer deps + fairness).
"""

import os
import numpy as np
import ml_dtypes
from contextlib import ExitStack

import concourse.bass as bass
import concourse.bacc as bacc
import concourse.tile as tile
from concourse import mybir
from concourse.bass_utils import run_bass_kernel_spmd
from concourse.masks import make_identity

P = 128
M, K, N = 8192, 4096, 4096
NCORES = 8
M_SHARD = M // NCORES          # 1024 rows of x per core
M_TILES = M_SHARD // P         # 8
K_SUBS = K // P                # 32
K_SUPERS = K // (2 * P)        # 16 (DoubleRow consumes 256 rows of K)
KH = K // 2                    # 2048, half-tile for split reduces
N_BLK = 512
N_BLKS = N // N_BLK            # 8
NB_PHASE1 = 2                  # GEMM N-blocks interleaved into the quant loop

FP8 = mybir.dt.float8e4
F32 = mybir.dt.float32
BF16 = mybir.dt.bfloat16

USE_IS_TRANSPOSE = True

_PROGRAM_CACHE = {}


def _build_program():
    nc = bacc.Bacc(None, target_bir_lowering=False)

    x_d = nc.declare_dram_parameter("x", [M_SHARD, K], BF16, isOutput=False)
    # host layout: wt[nb, p, ksub, n] = weight[nb*512 + n, ksub*128 + p],
    # losslessly re-encoded to fp8 (reference weights are fp8-round-tripped)
    wt_d = nc.declare_dram_parameter("wt", [N_BLKS, P, K_SUBS, N_BLK], FP8, isOutput=False)
    ws_d = nc.declare_dram_parameter("ws", [N], F32, isOutput=False)
    bias_d = nc.declare_dram_parameter("bias", [N], F32, isOutput=False)
    out_d = nc.declare_dram_parameter("out", [M_SHARD, N], BF16, isOutput=True)

    x_ap = x_d[:]
    wt_ap = wt_d[:]
    out_ap = out_d[:]

    with tile.TileContext(nc) as tc, ExitStack() as ctx:
        singles = ctx.enter_context(tc.tile_pool(name="singles", bufs=1))
        xpool = ctx.enter_context(tc.tile_pool(name="xpool", bufs=3))
        xqpool = ctx.enter_context(tc.tile_pool(name="xqpool", bufs=2))
        stats = ctx.enter_context(tc.tile_pool(name="stats", bufs=4))
        xspool = ctx.enter_context(tc.tile_pool(name="xspool", bufs=M_TILES))
        xqtpool = ctx.enter_context(tc.tile_pool(name="xqtpool", bufs=M_TILES))
        wqpool = ctx.enter_context(tc.tile_pool(name="wqpool", bufs=12))
        opool = ctx.enter_context(tc.tile_pool(name="opool", bufs=4))
        psum_tr = ctx.enter_context(tc.tile_pool(name="psum_tr", bufs=2, space="PSUM"))
        psum_mm = ctx.enter_context(tc.tile_pool(name="psum_mm", bufs=4, space="PSUM"))

        # ---- upfront DMA issue: x tiles 0-1 on the sync ring; weight slabs
        # on the scalar ring (first two quartered); ws/bias broadcasts are
        # HBM-read-light and use the scalar ring's broadcast path.
        x_tiles = [None] * M_TILES

        def issue_x(mt, split=False):
            t = xpool.tile([P, K], BF16, tag="xt")
            if split:
                # first tile rides both DGE rings so the halves land in
                # parallel right after ring warmup
                nc.sync.dma_start(out=t[:, 0:KH], in_=x_ap[mt * P:(mt + 1) * P, 0:KH])
                nc.scalar.dma_start(out=t[:, KH:K], in_=x_ap[mt * P:(mt + 1) * P, KH:K])
            else:
                nc.sync.dma_start(out=t[:], in_=x_ap[mt * P:(mt + 1) * P, :])
            x_tiles[mt] = t

        wslab_tiles = [None] * N_BLKS

        def issue_wslab(nb, engine):
            # every slab loads as 4 quarter-DMAs: 4KB-per-partition runs keep
            # the round-robin DMA queues fair vs the 8KB x rows, and give the
            # GEMM quarter-granular deps on the arriving weights
            quarters = []
            for q in range(4):
                t = wqpool.tile([P, 8, N_BLK], FP8, tag="wq")
                engine.dma_start(out=t[:], in_=wt_ap[nb][:, 8 * q:8 * q + 8, :])
                quarters.append(t)
            wslab_tiles[nb] = quarters

        def slab_rhs(nb, j):
            jj = j % 4
            return wslab_tiles[nb][j // 4][:, 2 * jj:2 * jj + 2, :]

        # all bulk DMAs ride the sync ring: doorbells block when the DGE ring
        # fills, and the scalar ring's doorbells would stall ACT compute
        # queued behind them
        issue_x(0, split=True)
        issue_x(1)
        issue_wslab(0, nc.sync)

        ident = singles.tile([P, P], FP8)
        make_identity(nc, ident)

        # ws/bias load as 16KB rows and broadcast on-chip via the idle
        # gpsimd engine: a 2MB DMA row-broadcast completes slowly and its
        # doorbell would occupy the shallow (~3-deep) scalar DGE ring
        ws_row = singles.tile([1, N], F32)
        nc.scalar.dma_start(out=ws_row[:], in_=bass.AP(tensor=ws_d[:].tensor, offset=0, ap=[[0, 1], [1, N]]))
        bias_row = singles.tile([1, N], F32)
        nc.scalar.dma_start(out=bias_row[:], in_=bass.AP(tensor=bias_d[:].tensor, offset=0, ap=[[0, 1], [1, N]]))
        # slab1 rides the otherwise-idle scalar ring; its doorbells clear the
        # DGE ring just before the first quant needs the ACT queue
        issue_wslab(1, nc.scalar)
        ws_b = singles.tile([P, N], F32)
        nc.gpsimd.partition_broadcast(ws_b[:], ws_row[:], channels=P)
        bias_b = singles.tile([P, N], F32)
        nc.gpsimd.partition_broadcast(bias_b[:], bias_row[:], channels=P)

        xs_tiles = []
        xqt_tiles = []
        prev_inv_inst = None

        def epilogue(mt, nb, pm, phase1):
            # out = bf16(psum * xs[m] * ws[n]) + bias[n]; the fused
            # scalar_tensor_tensor keeps a single rounding to bf16.
            sb1 = opool.tile([P, N_BLK], BF16, tag="sb1")
            nc.vector.scalar_tensor_tensor(
                out=sb1[:], in0=pm[:], scalar=xs_tiles[mt][:],
                in1=ws_b[:, nb * N_BLK:(nb + 1) * N_BLK],
                op0=mybir.AluOpType.mult, op1=mybir.AluOpType.mult,
            )
            sb2 = opool.tile([P, N_BLK], BF16, tag="sb2")
            eng = nc.gpsimd if phase1 else nc.vector
            eng.tensor_add(sb2[:], sb1[:], bias_b[:, nb * N_BLK:(nb + 1) * N_BLK])
            nc.sync.dma_start(
                out=out_ap[mt * P:(mt + 1) * P, nb * N_BLK:(nb + 1) * N_BLK],
                in_=sb2[:],
            )

        def gemm_block(mt, nb, phase1=False):
            pm = psum_mm.tile([P, N_BLK], F32, tag="pm")
            for j in range(K_SUPERS):
                g, jj = divmod(j, 4)
                nc.tensor.matmul(
                    out=pm[:],
                    lhsT=xqt_tiles[mt][g][:, 2 * jj:2 * jj + 2, :],
                    rhs=slab_rhs(nb, j),
                    start=(j == 0), stop=(j == K_SUPERS - 1),
                    perf_mode=mybir.MatmulPerfMode.DoubleRow,
                )
            epilogue(mt, nb, pm, phase1)

        # ---- phase 1: per 128-row tile: quantize, transpose, and two
        # N-blocks of GEMM (keeps the PE saturated while later tiles
        # quantize).
        for mt in range(M_TILES):
            if mt + 2 < M_TILES:
                issue_x(mt + 2)
            if mt == 2:
                issue_wslab(2, nc.sync)

            xt = x_tiles[mt]
            amax_a = stats.tile([P, 1], F32, tag="amax_a")
            reduce_a = nc.vector.tensor_reduce(
                out=amax_a[:], in_=xt[:, 0:KH],
                axis=mybir.AxisListType.X, op=mybir.AluOpType.max,
                apply_absolute_value=True,
            )
            amax_b = stats.tile([P, 1], F32, tag="amax_b")
            reduce_b = nc.vector.tensor_reduce(
                out=amax_b[:], in_=xt[:, KH:K],
                axis=mybir.AxisListType.X, op=mybir.AluOpType.max,
                apply_absolute_value=True,
            )
            # order this tile's reduces after the previous tile's scale
            # chain so the 2.3us reduces don't delay the chain that gates
            # ACT quant
            if prev_inv_inst is not None:
                tile.add_dep_helper(reduce_a.ins, prev_inv_inst.ins, sync=False,
                                    reason="stats chain before next reduce")
                tile.add_dep_helper(reduce_b.ins, prev_inv_inst.ins, sync=False,
                                    reason="stats chain before next reduce")
            with tc.high_priority():
                amax = stats.tile([P, 1], F32, tag="amax")
                nc.vector.tensor_max(amax[:], amax_a[:], amax_b[:])
                # xs = max(amax, eps) * (1/224); quant scale is exactly 1/xs
                xs = xspool.tile([P, 1], F32, tag="xs")
                nc.vector.tensor_scalar(
                    out=xs[:], in0=amax[:],
                    scalar1=1e-10, scalar2=1.0 / 224.0,
                    op0=mybir.AluOpType.max, op1=mybir.AluOpType.mult,
                )
                xs_tiles.append(xs)
                inv = stats.tile([P, 1], F32, tag="inv")
                prev_inv_inst = nc.vector.reciprocal(out=inv[:], in_=xs[:])

            # quantize in halves: the transposes of half A start while
            # half B is still quantizing on ACT
            xq_parts = []
            for h in range(2):
                xq_h = xqpool.tile([P, KH], FP8, tag=f"xq{h}")
                nc.scalar.activation(
                    out=xq_h[:], in_=xt[:, h * KH:(h + 1) * KH],
                    func=mybir.ActivationFunctionType.Copy, scale=inv[:],
                )
                xq_parts.append(xq_h)

            def xq_chunk(ks):
                return xq_parts[ks // 16][:, (ks % 16) * P:(ks % 16 + 1) * P]

            # transpose x_q into [K, M] layout via PE transpose matmuls;
            # evict each 8-ksub group right after its matmuls so the GEMM's
            # j=0..3 can start as soon as the first group lands in SBUF
            xqt_groups = []
            for half in range(2):
                if USE_IS_TRANSPOSE:
                    # fp8 transpose mode writes elements on a 2-byte step
                    ptr = psum_tr.tile([P, 16, 2 * P], FP8, tag="ptr")
                    ptr_view = ptr[:, :, 0:2 * P:2]
                else:
                    ptr = psum_tr.tile([P, 16, P], F32, tag="ptr")
                    ptr_view = ptr[:]
                for g in range(2):
                    for i in range(8):
                        nc.tensor.matmul(
                            out=ptr_view[:, 8 * g + i, :],
                            lhsT=xq_chunk(half * 16 + 8 * g + i),
                            rhs=ident[:],
                            start=True, stop=True,
                            is_transpose=USE_IS_TRANSPOSE,
                        )
                    xqt_g = xqtpool.tile([P, 8, P], FP8, tag=f"xqt{2 * half + g}")
                    xqt_groups.append(xqt_g)
                    nc.scalar.copy(out=xqt_g[:], in_=ptr_view[:, 8 * g:8 * g + 8, :])
            xqt_tiles.append(xqt_groups)

            if mt >= 1:
                gemm_block(mt - 1, 1, phase1=True)
            gemm_block(mt, 0, phase1=True)

        gemm_block(M_TILES - 1, 1, phase1=True)

        # ---- phase 2: pure fp8 DoubleRow GEMM over the remaining N-blocks
        for nb in range(NB_PHASE1, N_BLKS):
            if nb + 1 < N_BLKS:
                issue_wslab(nb + 1, nc.sync)
            for mt in range(M_TILES):
                gemm_block(mt, nb)

    nc.compile()
    return nc


def _get_program():
    if "nc" not in _PROGRAM_CACHE:
        _PROGRAM_CACHE["nc"] = _build_program()
    return _PROGRAM_CACHE["nc"]


def _run_sharded(x, weight, weight_scales, bias, trace=False):
    x = np.asarray(x).astype(ml_dtypes.bfloat16, copy=False)
    weight = np.asarray(weight, dtype=np.float32)
    weight_scales = np.asarray(weight_scales, dtype=np.float32)
    bias = np.asarray(bias, dtype=np.float32)

    # host-side sharding / layout only:
    # wt[nb, p, ksub, n] = weight[nb*512 + n, ksub*128 + p], re-encoded to
    # fp8 e4m3 (lossless: the reference weights are fp8-round-tripped values)
    wt = np.ascontiguousarray(
        weight.T.reshape(K_SUBS, P, N_BLKS, N_BLK).transpose(2, 1, 0, 3)
    ).astype(ml_dtypes.float8_e4m3)
    in_maps = []
    for c in range(NCORES):
        in_maps.append({
            "x": np.ascontiguousarray(x[c * M_SHARD:(c + 1) * M_SHARD]),
            "wt": wt,
            "ws": weight_scales,
            "bias": bias,
        })

    nc = _get_program()
    res = run_bass_kernel_spmd(nc, in_maps, core_ids=list(range(NCORES)), trace=trace)
    out = np.concatenate([res.results[c]["out"] for c in range(NCORES)], axis=0)
    return out, res.exec_time_ns


def kernel(x, weight, weight_scales, bias):
    out, _ = _run_sharded(x, weight, weight_scales, bias,
                          trace=bool(os.environ.get("KERNEL_TRACE")))
    return out


# revision 27
# speedup vs baseline: 1.0732x; 1.0008x over previous
"""Fp8 per-token/per-channel quantized linear for Trainium2, 8 NeuronCores.

Computation (matches the jax reference):
    amax[m]  = max_k |x[m, k]|                       (x is bf16)
    xs[m]    = max(amax, 1e-10) / 448
    x_q      = e4m3fn_round(x / xs)                  (values up to +-448)
    out      = bf16((x_q @ W^T) * xs * w_scales) + bf16(bias)

Mapping to TRN2 hardware:
  * TRN's fp8 E4M3 saturates at +-240 (256..448 are Inf/NaN), so we quantize
    at HALF scale: x_q' = e4m3_round(x * (224/amax)) == x_q / 2 exactly (the
    fp8 grid is self-similar under powers of two), and fold the factor 2 into
    the output scale: out = psum * (amax/224) * w_scales.  The reference
    weights are already exactly fp8-representable, so casting them is lossless.
  * Sharding: row-parallel over M (8 cores x 1024 rows).  Each core quantizes
    only its own rows, and streams the full weight, transposed on host to
    [K, N] tile layout and losslessly re-encoded to fp8.
  * x_q is transposed on-chip into [K, M] layout with PE transpose matmuls
    (contraction must sit on partitions for both matmul operands).
  * Main GEMM runs in fp8 with perf_mode=DoubleRow (k=256 per matmul).

Schedule (v4): the kernel is PE-bound (DoubleRow GEMM ~221us + transposes
~25us).  The per-tile producer chain is DVE amax (f32 reduce, no fast DVE
mode exists: ~4.6us) + tiny scale chain, ACT quant copy (~3.7us) and two
ACT psum evicts (~4.1us); DVE additionally runs the fused epilogue
(psum*xs*ws in one scalar_tensor_tensor).  ACT (~7.9us) and DVE (~6.6us)
both stay under the PE's 8.6us per-tile consumption of T(mt) + GEMM(mt,0)
+ GEMM(mt,1), so phase 1 runs PE-bound; phase 2 is pure GEMM.  DMA queue
fairness is round-robin per ~descriptor, so x tiles load as single [128,4096]
DMAs (8KB runs) on the sync ring while weight slabs ride the scalar ring,
the first two split into 4KB-run quarters (finer deps + fairness).
"""

import os
import numpy as np
import ml_dtypes
from contextlib import ExitStack

import concourse.bass as bass
import concourse.bacc as bacc
import concourse.tile as tile
from concourse import mybir
from concourse.bass_utils import run_bass_kernel_spmd
from concourse.masks import make_identity

P = 128
M, K, N = 8192, 4096, 4096
NCORES = 8
M_SHARD = M // NCORES          # 1024 rows of x per core
M_TILES = M_SHARD // P         # 8
K_SUBS = K // P                # 32
K_SUPERS = K // (2 * P)        # 16 (DoubleRow consumes 256 rows of K)
KH = K // 2                    # 2048, half-tile for split reduces
N_BLK = 512
N_BLKS = N // N_BLK            # 8
NB_PHASE1 = 2                  # GEMM N-blocks interleaved into the quant loop

FP8 = mybir.dt.float8e4
F32 = mybir.dt.float32
BF16 = mybir.dt.bfloat16

USE_IS_TRANSPOSE = True

_PROGRAM_CACHE = {}


def _build_program():
    nc = bacc.Bacc(None, target_bir_lowering=False)

    x_d = nc.declare_dram_parameter("x", [M_SHARD, K], BF16, isOutput=False)
    # host layout: wt[nb, p, ksub, n] = weight[nb*512 + n, ksub*128 + p],
    # losslessly re-encoded to fp8 (reference weights are fp8-round-tripped)
    wt_d = nc.declare_dram_parameter("wt", [N_BLKS, P, K_SUBS, N_BLK], FP8, isOutput=False)
    ws_d = nc.declare_dram_parameter("ws", [N], F32, isOutput=False)
    bias_d = nc.declare_dram_parameter("bias", [N], F32, isOutput=False)
    out_d = nc.declare_dram_parameter("out", [M_SHARD, N], BF16, isOutput=True)

    x_ap = x_d[:]
    wt_ap = wt_d[:]
    out_ap = out_d[:]

    with tile.TileContext(nc) as tc, ExitStack() as ctx:
        singles = ctx.enter_context(tc.tile_pool(name="singles", bufs=1))
        xpool = ctx.enter_context(tc.tile_pool(name="xpool", bufs=3))
        xqpool = ctx.enter_context(tc.tile_pool(name="xqpool", bufs=2))
        stats = ctx.enter_context(tc.tile_pool(name="stats", bufs=4))
        xspool = ctx.enter_context(tc.tile_pool(name="xspool", bufs=M_TILES))
        xqtpool = ctx.enter_context(tc.tile_pool(name="xqtpool", bufs=M_TILES))
        wqpool = ctx.enter_context(tc.tile_pool(name="wqpool", bufs=12))
        opool = ctx.enter_context(tc.tile_pool(name="opool", bufs=4))
        psum_tr = ctx.enter_context(tc.tile_pool(name="psum_tr", bufs=2, space="PSUM"))
        psum_mm = ctx.enter_context(tc.tile_pool(name="psum_mm", bufs=4, space="PSUM"))

        # ---- upfront DMA issue: x tiles 0-1 on the sync ring; weight slabs
        # on the scalar ring (first two quartered); ws/bias broadcasts are
        # HBM-read-light and use the scalar ring's broadcast path.
        x_tiles = [None] * M_TILES

        def issue_x(mt, split=False):
            t = xpool.tile([P, K], BF16, tag="xt")
            if split:
                # first tile rides both DGE rings so the halves land in
                # parallel right after ring warmup
                nc.sync.dma_start(out=t[:, 0:KH], in_=x_ap[mt * P:(mt + 1) * P, 0:KH])
                nc.scalar.dma_start(out=t[:, KH:K], in_=x_ap[mt * P:(mt + 1) * P, KH:K])
            else:
                nc.sync.dma_start(out=t[:], in_=x_ap[mt * P:(mt + 1) * P, :])
            x_tiles[mt] = t

        wslab_tiles = [None] * N_BLKS

        def issue_wslab(nb, engine):
            # every slab loads as 4 quarter-DMAs: 4KB-per-partition runs keep
            # the round-robin DMA queues fair vs the 8KB x rows, and give the
            # GEMM quarter-granular deps on the arriving weights
            quarters = []
            for q in range(4):
                t = wqpool.tile([P, 8, N_BLK], FP8, tag="wq")
                engine.dma_start(out=t[:], in_=wt_ap[nb][:, 8 * q:8 * q + 8, :])
                quarters.append(t)
            wslab_tiles[nb] = quarters

        def slab_rhs(nb, j):
            jj = j % 4
            return wslab_tiles[nb][j // 4][:, 2 * jj:2 * jj + 2, :]

        # DGE rings retire in order and doorbells block while the ring is
        # full, so small row-loads go first on the scalar ring; bulk loads
        # are ordered by deadline.  ws/bias broadcast on-chip via the idle
        # gpsimd engine instead of a slow 2MB DMA row-broadcast.
        ws_row = singles.tile([1, N], F32)
        nc.scalar.dma_start(out=ws_row[:], in_=bass.AP(tensor=ws_d[:].tensor, offset=0, ap=[[0, 1], [1, N]]))
        bias_row = singles.tile([1, N], F32)
        nc.scalar.dma_start(out=bias_row[:], in_=bass.AP(tensor=bias_d[:].tensor, offset=0, ap=[[0, 1], [1, N]]))

        xt0 = xpool.tile([P, K], BF16, tag="xt")
        nc.sync.dma_start(out=xt0[:, 0:KH], in_=x_ap[0:P, 0:KH])
        nc.scalar.dma_start(out=xt0[:, KH:K], in_=x_ap[0:P, KH:K])
        x_tiles[0] = xt0
        issue_wslab(1, nc.scalar)

        sq0 = [wqpool.tile([P, 8, N_BLK], FP8, tag="wq", name=f"s0q{q}") for q in range(4)]
        for q in (0, 1):
            nc.sync.dma_start(out=sq0[q][:], in_=wt_ap[0][:, 8 * q:8 * q + 8, :])
        issue_x(1)
        for q in (2, 3):
            nc.sync.dma_start(out=sq0[q][:], in_=wt_ap[0][:, 8 * q:8 * q + 8, :])
        wslab_tiles[0] = sq0

        ident = singles.tile([P, P], FP8)
        make_identity(nc, ident)
        ws_b = singles.tile([P, N], F32)
        nc.gpsimd.partition_broadcast(ws_b[:], ws_row[:], channels=P)
        bias_b = singles.tile([P, N], F32)
        nc.gpsimd.partition_broadcast(bias_b[:], bias_row[:], channels=P)

        xs_tiles = []
        xqt_tiles = []
        prev_inv_inst = None

        def epilogue(mt, nb, pm, phase1):
            # out = bf16(psum * xs[m] * ws[n]) + bias[n]; the fused
            # scalar_tensor_tensor keeps a single rounding to bf16.
            sb1 = opool.tile([P, N_BLK], BF16, tag="sb1")
            nc.vector.scalar_tensor_tensor(
                out=sb1[:], in0=pm[:], scalar=xs_tiles[mt][:],
                in1=ws_b[:, nb * N_BLK:(nb + 1) * N_BLK],
                op0=mybir.AluOpType.mult, op1=mybir.AluOpType.mult,
            )
            sb2 = opool.tile([P, N_BLK], BF16, tag="sb2")
            eng = nc.gpsimd if phase1 else nc.vector
            eng.tensor_add(sb2[:], sb1[:], bias_b[:, nb * N_BLK:(nb + 1) * N_BLK])
            nc.sync.dma_start(
                out=out_ap[mt * P:(mt + 1) * P, nb * N_BLK:(nb + 1) * N_BLK],
                in_=sb2[:],
            )

        def gemm_block(mt, nb, phase1=False):
            pm = psum_mm.tile([P, N_BLK], F32, tag="pm")
            for j in range(K_SUPERS):
                g, jj = divmod(j, 4)
                nc.tensor.matmul(
                    out=pm[:],
                    lhsT=xqt_tiles[mt][g][:, 2 * jj:2 * jj + 2, :],
                    rhs=slab_rhs(nb, j),
                    start=(j == 0), stop=(j == K_SUPERS - 1),
                    perf_mode=mybir.MatmulPerfMode.DoubleRow,
                )
            epilogue(mt, nb, pm, phase1)

        # ---- phase 1: per 128-row tile: quantize, transpose, and two
        # N-blocks of GEMM (keeps the PE saturated while later tiles
        # quantize).
        for mt in range(M_TILES):
            if mt + 2 < M_TILES:
                issue_x(mt + 2)
            if mt == 5:
                issue_wslab(2, nc.sync)

            xt = x_tiles[mt]
            amax_a = stats.tile([P, 1], F32, tag="amax_a")
            reduce_a = nc.vector.tensor_reduce(
                out=amax_a[:], in_=xt[:, 0:KH],
                axis=mybir.AxisListType.X, op=mybir.AluOpType.max,
                apply_absolute_value=True,
            )
            amax_b = stats.tile([P, 1], F32, tag="amax_b")
            reduce_b = nc.vector.tensor_reduce(
                out=amax_b[:], in_=xt[:, KH:K],
                axis=mybir.AxisListType.X, op=mybir.AluOpType.max,
                apply_absolute_value=True,
            )
            # order this tile's reduces after the previous tile's scale
            # chain so the 2.3us reduces don't delay the chain that gates
            # ACT quant
            if prev_inv_inst is not None:
                tile.add_dep_helper(reduce_a.ins, prev_inv_inst.ins, sync=False,
                                    reason="stats chain before next reduce")
                tile.add_dep_helper(reduce_b.ins, prev_inv_inst.ins, sync=False,
                                    reason="stats chain before next reduce")
            with tc.high_priority():
                amax = stats.tile([P, 1], F32, tag="amax")
                nc.vector.tensor_max(amax[:], amax_a[:], amax_b[:])
                # xs = max(amax, eps) * (1/224); quant scale is exactly 1/xs
                xs = xspool.tile([P, 1], F32, tag="xs")
                nc.vector.tensor_scalar(
                    out=xs[:], in0=amax[:],
                    scalar1=1e-10, scalar2=1.0 / 224.0,
                    op0=mybir.AluOpType.max, op1=mybir.AluOpType.mult,
                )
                xs_tiles.append(xs)
                inv = stats.tile([P, 1], F32, tag="inv")
                prev_inv_inst = nc.vector.reciprocal(out=inv[:], in_=xs[:])

            # quantize in halves: the transposes of half A start while
            # half B is still quantizing on ACT
            xq_parts = []
            for h in range(2):
                xq_h = xqpool.tile([P, KH], FP8, tag=f"xq{h}")
                nc.scalar.activation(
                    out=xq_h[:], in_=xt[:, h * KH:(h + 1) * KH],
                    func=mybir.ActivationFunctionType.Copy, scale=inv[:],
                )
                xq_parts.append(xq_h)

            def xq_chunk(ks):
                return xq_parts[ks // 16][:, (ks % 16) * P:(ks % 16 + 1) * P]

            # transpose x_q into [K, M] layout via PE transpose matmuls;
            # evict each 8-ksub group right after its matmuls so the GEMM's
            # j=0..3 can start as soon as the first group lands in SBUF
            xqt_groups = []
            for half in range(2):
                if USE_IS_TRANSPOSE:
                    # fp8 transpose mode writes elements on a 2-byte step
                    ptr = psum_tr.tile([P, 16, 2 * P], FP8, tag="ptr")
                    ptr_view = ptr[:, :, 0:2 * P:2]
                else:
                    ptr = psum_tr.tile([P, 16, P], F32, tag="ptr")
                    ptr_view = ptr[:]
                for g in range(2):
                    for i in range(8):
                        nc.tensor.matmul(
                            out=ptr_view[:, 8 * g + i, :],
                            lhsT=xq_chunk(half * 16 + 8 * g + i),
                            rhs=ident[:],
                            start=True, stop=True,
                            is_transpose=USE_IS_TRANSPOSE,
                        )
                    xqt_g = xqtpool.tile([P, 8, P], FP8, tag=f"xqt{2 * half + g}")
                    xqt_groups.append(xqt_g)
                    nc.scalar.copy(out=xqt_g[:], in_=ptr_view[:, 8 * g:8 * g + 8, :])
            xqt_tiles.append(xqt_groups)

            if mt >= 1:
                gemm_block(mt - 1, 1, phase1=True)
            gemm_block(mt, 0, phase1=True)

        gemm_block(M_TILES - 1, 1, phase1=True)

        # ---- phase 2: pure fp8 DoubleRow GEMM over the remaining N-blocks
        for nb in range(NB_PHASE1, N_BLKS):
            if nb + 1 < N_BLKS:
                issue_wslab(nb + 1, nc.sync)
            for mt in range(M_TILES):
                gemm_block(mt, nb)

    nc.compile()
    return nc


def _get_program():
    if "nc" not in _PROGRAM_CACHE:
        _PROGRAM_CACHE["nc"] = _build_program()
    return _PROGRAM_CACHE["nc"]


def _run_sharded(x, weight, weight_scales, bias, trace=False):
    x = np.asarray(x).astype(ml_dtypes.bfloat16, copy=False)
    weight = np.asarray(weight, dtype=np.float32)
    weight_scales = np.asarray(weight_scales, dtype=np.float32)
    bias = np.asarray(bias, dtype=np.float32)

    # host-side sharding / layout only:
    # wt[nb, p, ksub, n] = weight[nb*512 + n, ksub*128 + p], re-encoded to
    # fp8 e4m3 (lossless: the reference weights are fp8-round-tripped values)
    wt = np.ascontiguousarray(
        weight.T.reshape(K_SUBS, P, N_BLKS, N_BLK).transpose(2, 1, 0, 3)
    ).astype(ml_dtypes.float8_e4m3)
    in_maps = []
    for c in range(NCORES):
        in_maps.append({
            "x": np.ascontiguousarray(x[c * M_SHARD:(c + 1) * M_SHARD]),
            "wt": wt,
            "ws": weight_scales,
            "bias": bias,
        })

    nc = _get_program()
    res = run_bass_kernel_spmd(nc, in_maps, core_ids=list(range(NCORES)), trace=trace)
    out = np.concatenate([res.results[c]["out"] for c in range(NCORES)], axis=0)
    return out, res.exec_time_ns


def kernel(x, weight, weight_scales, bias):
    out, _ = _run_sharded(x, weight, weight_scales, bias,
                          trace=bool(os.environ.get("KERNEL_TRACE")))
    return out


# revision 28
# speedup vs baseline: 1.0920x; 1.0175x over previous
"""Fp8 per-token/per-channel quantized linear for Trainium2, 8 NeuronCores.

Computation (matches the jax reference):
    amax[m]  = max_k |x[m, k]|                       (x is bf16)
    xs[m]    = max(amax, 1e-10) / 448
    x_q      = e4m3fn_round(x / xs)                  (values up to +-448)
    out      = bf16((x_q @ W^T) * xs * w_scales) + bf16(bias)

Mapping to TRN2 hardware:
  * TRN's fp8 E4M3 saturates at +-240 (256..448 are Inf/NaN), so we quantize
    at HALF scale: x_q' = e4m3_round(x * (224/amax)) == x_q / 2 exactly (the
    fp8 grid is self-similar under powers of two), and fold the factor 2 into
    the output scale: out = psum * (amax/224) * w_scales.  The reference
    weights are already exactly fp8-representable, so casting them is lossless.
  * Sharding: row-parallel over M (8 cores x 1024 rows).  Each core quantizes
    only its own rows, and streams the full weight, transposed on host to
    [K, N] tile layout and losslessly re-encoded to fp8.
  * x_q is transposed on-chip into [K, M] layout with PE transpose matmuls
    (contraction must sit on partitions for both matmul operands).
  * Main GEMM runs in fp8 with perf_mode=DoubleRow (k=256 per matmul).

Schedule (v4): the kernel is PE-bound (DoubleRow GEMM ~221us + transposes
~25us).  The per-tile producer chain is DVE amax (f32 reduce, no fast DVE
mode exists: ~4.6us) + tiny scale chain, ACT quant copy (~3.7us) and two
ACT psum evicts (~4.1us); DVE additionally runs the fused epilogue
(psum*xs*ws in one scalar_tensor_tensor).  ACT (~7.9us) and DVE (~6.6us)
both stay under the PE's 8.6us per-tile consumption of T(mt) + GEMM(mt,0)
+ GEMM(mt,1), so phase 1 runs PE-bound; phase 2 is pure GEMM.  DMA queue
fairness is round-robin per ~descriptor, so x tiles load as single [128,4096]
DMAs (8KB runs) on the sync ring while weight slabs ride the scalar ring,
the first two split into 4KB-run quarters (finer deps + fairness).
"""

import os
import numpy as np
import ml_dtypes
from contextlib import ExitStack

import concourse.bass as bass
import concourse.bacc as bacc
import concourse.tile as tile
from concourse import mybir
from concourse.bass_utils import run_bass_kernel_spmd
from concourse.masks import make_identity

P = 128
M, K, N = 8192, 4096, 4096
NCORES = 8
M_SHARD = M // NCORES          # 1024 rows of x per core
M_TILES = M_SHARD // P         # 8
K_SUBS = K // P                # 32
K_SUPERS = K // (2 * P)        # 16 (DoubleRow consumes 256 rows of K)
KH = K // 2                    # 2048, half-tile for split reduces
N_BLK = 512
N_BLKS = N // N_BLK            # 8
NB_PHASE1 = 2                  # GEMM N-blocks interleaved into the quant loop

FP8 = mybir.dt.float8e4
F32 = mybir.dt.float32
BF16 = mybir.dt.bfloat16

USE_IS_TRANSPOSE = True

_PROGRAM_CACHE = {}


def _build_program():
    nc = bacc.Bacc(None, target_bir_lowering=False)

    x_d = nc.declare_dram_parameter("x", [M_SHARD, K], BF16, isOutput=False)
    # host layout: wt[nb, p, ksub, n] = weight[nb*512 + n, ksub*128 + p],
    # losslessly re-encoded to fp8 (reference weights are fp8-round-tripped)
    wt_d = nc.declare_dram_parameter("wt", [N_BLKS, P, K_SUBS, N_BLK], FP8, isOutput=False)
    ws_d = nc.declare_dram_parameter("ws", [N], F32, isOutput=False)
    bias_d = nc.declare_dram_parameter("bias", [N], F32, isOutput=False)
    out_d = nc.declare_dram_parameter("out", [M_SHARD, N], BF16, isOutput=True)

    x_ap = x_d[:]
    wt_ap = wt_d[:]
    out_ap = out_d[:]

    with tile.TileContext(nc) as tc, ExitStack() as ctx:
        singles = ctx.enter_context(tc.tile_pool(name="singles", bufs=1))
        xpool = ctx.enter_context(tc.tile_pool(name="xpool", bufs=3))
        xqpool = ctx.enter_context(tc.tile_pool(name="xqpool", bufs=2))
        stats = ctx.enter_context(tc.tile_pool(name="stats", bufs=4))
        xspool = ctx.enter_context(tc.tile_pool(name="xspool", bufs=M_TILES))
        xqtpool = ctx.enter_context(tc.tile_pool(name="xqtpool", bufs=M_TILES))
        wqpool = ctx.enter_context(tc.tile_pool(name="wqpool", bufs=12))
        opool = ctx.enter_context(tc.tile_pool(name="opool", bufs=4))
        psum_tr = ctx.enter_context(tc.tile_pool(name="psum_tr", bufs=2, space="PSUM"))
        psum_mm = ctx.enter_context(tc.tile_pool(name="psum_mm", bufs=4, space="PSUM"))

        # ---- upfront DMA issue: x tiles 0-1 on the sync ring; weight slabs
        # on the scalar ring (first two quartered); ws/bias broadcasts are
        # HBM-read-light and use the scalar ring's broadcast path.
        x_tiles = [None] * M_TILES

        def issue_x(mt, split=False):
            t = xpool.tile([P, K], BF16, tag="xt")
            if split:
                # first tile rides both DGE rings so the halves land in
                # parallel right after ring warmup
                nc.sync.dma_start(out=t[:, 0:KH], in_=x_ap[mt * P:(mt + 1) * P, 0:KH])
                nc.scalar.dma_start(out=t[:, KH:K], in_=x_ap[mt * P:(mt + 1) * P, KH:K])
            else:
                nc.sync.dma_start(out=t[:], in_=x_ap[mt * P:(mt + 1) * P, :])
            x_tiles[mt] = t

        wslab_tiles = [None] * N_BLKS

        def issue_wslab(nb, engine):
            # every slab loads as 4 quarter-DMAs: 4KB-per-partition runs keep
            # the round-robin DMA queues fair vs the 8KB x rows, and give the
            # GEMM quarter-granular deps on the arriving weights
            quarters = []
            for q in range(4):
                t = wqpool.tile([P, 8, N_BLK], FP8, tag="wq")
                engine.dma_start(out=t[:], in_=wt_ap[nb][:, 8 * q:8 * q + 8, :])
                quarters.append(t)
            wslab_tiles[nb] = quarters

        def slab_rhs(nb, j):
            jj = j % 4
            return wslab_tiles[nb][j // 4][:, 2 * jj:2 * jj + 2, :]

        # DGE rings retire in order and doorbells block while the ring is
        # full, so small row-loads go first on the scalar ring; bulk loads
        # are ordered by deadline.  ws/bias broadcast on-chip via the idle
        # gpsimd engine instead of a slow 2MB DMA row-broadcast.
        xt0 = xpool.tile([P, K], BF16, tag="xt")
        nc.sync.dma_start(out=xt0[:, 0:KH], in_=x_ap[0:P, 0:KH])
        nc.scalar.dma_start(out=xt0[:, KH:K], in_=x_ap[0:P, KH:K])
        x_tiles[0] = xt0
        ws_row = singles.tile([1, N], F32)
        nc.scalar.dma_start(out=ws_row[:], in_=bass.AP(tensor=ws_d[:].tensor, offset=0, ap=[[0, 1], [1, N]]))
        bias_row = singles.tile([1, N], F32)
        nc.scalar.dma_start(out=bias_row[:], in_=bass.AP(tensor=bias_d[:].tensor, offset=0, ap=[[0, 1], [1, N]]))
        issue_wslab(1, nc.scalar)

        sq0 = [wqpool.tile([P, 8, N_BLK], FP8, tag="wq", name=f"s0q{q}") for q in range(4)]
        for q in (0, 1):
            nc.sync.dma_start(out=sq0[q][:], in_=wt_ap[0][:, 8 * q:8 * q + 8, :])
        issue_x(1)
        for q in (2, 3):
            nc.sync.dma_start(out=sq0[q][:], in_=wt_ap[0][:, 8 * q:8 * q + 8, :])
        wslab_tiles[0] = sq0

        ident = singles.tile([P, P], FP8)
        make_identity(nc, ident)
        ws_b = singles.tile([P, N], F32)
        nc.gpsimd.partition_broadcast(ws_b[:], ws_row[:], channels=P)
        bias_b = singles.tile([P, N], F32)
        nc.gpsimd.partition_broadcast(bias_b[:], bias_row[:], channels=P)

        xs_tiles = []
        xqt_tiles = []
        prev_inv_inst = None

        def epilogue(mt, nb, pm, phase1):
            # out = bf16(psum * xs[m] * ws[n]) + bias[n]; the fused
            # scalar_tensor_tensor keeps a single rounding to bf16.
            sb1 = opool.tile([P, N_BLK], BF16, tag="sb1")
            nc.vector.scalar_tensor_tensor(
                out=sb1[:], in0=pm[:], scalar=xs_tiles[mt][:],
                in1=ws_b[:, nb * N_BLK:(nb + 1) * N_BLK],
                op0=mybir.AluOpType.mult, op1=mybir.AluOpType.mult,
            )
            sb2 = opool.tile([P, N_BLK], BF16, tag="sb2")
            eng = nc.gpsimd if phase1 else nc.vector
            eng.tensor_add(sb2[:], sb1[:], bias_b[:, nb * N_BLK:(nb + 1) * N_BLK])
            nc.sync.dma_start(
                out=out_ap[mt * P:(mt + 1) * P, nb * N_BLK:(nb + 1) * N_BLK],
                in_=sb2[:],
            )

        def gemm_block(mt, nb, phase1=False):
            pm = psum_mm.tile([P, N_BLK], F32, tag="pm")
            for j in range(K_SUPERS):
                g, jj = divmod(j, 4)
                nc.tensor.matmul(
                    out=pm[:],
                    lhsT=xqt_tiles[mt][g][:, 2 * jj:2 * jj + 2, :],
                    rhs=slab_rhs(nb, j),
                    start=(j == 0), stop=(j == K_SUPERS - 1),
                    perf_mode=mybir.MatmulPerfMode.DoubleRow,
                )
            epilogue(mt, nb, pm, phase1)

        # ---- phase 1: per 128-row tile: quantize, transpose, and two
        # N-blocks of GEMM (keeps the PE saturated while later tiles
        # quantize).
        for mt in range(M_TILES):
            if mt + 2 < M_TILES:
                issue_x(mt + 2)
            if mt == 5:
                issue_wslab(2, nc.sync)

            xt = x_tiles[mt]
            amax_a = stats.tile([P, 1], F32, tag="amax_a")
            reduce_a = nc.vector.tensor_reduce(
                out=amax_a[:], in_=xt[:, 0:KH],
                axis=mybir.AxisListType.X, op=mybir.AluOpType.max,
                apply_absolute_value=True,
            )
            amax_b = stats.tile([P, 1], F32, tag="amax_b")
            reduce_b = nc.vector.tensor_reduce(
                out=amax_b[:], in_=xt[:, KH:K],
                axis=mybir.AxisListType.X, op=mybir.AluOpType.max,
                apply_absolute_value=True,
            )
            # order this tile's reduces after the previous tile's scale
            # chain so the 2.3us reduces don't delay the chain that gates
            # ACT quant
            if prev_inv_inst is not None:
                tile.add_dep_helper(reduce_a.ins, prev_inv_inst.ins, sync=False,
                                    reason="stats chain before next reduce")
                tile.add_dep_helper(reduce_b.ins, prev_inv_inst.ins, sync=False,
                                    reason="stats chain before next reduce")
            with tc.high_priority():
                amax = stats.tile([P, 1], F32, tag="amax")
                nc.vector.tensor_max(amax[:], amax_a[:], amax_b[:])
                # xs = max(amax, eps) * (1/224); quant scale is exactly 1/xs
                xs = xspool.tile([P, 1], F32, tag="xs")
                nc.vector.tensor_scalar(
                    out=xs[:], in0=amax[:],
                    scalar1=1e-10, scalar2=1.0 / 224.0,
                    op0=mybir.AluOpType.max, op1=mybir.AluOpType.mult,
                )
                xs_tiles.append(xs)
                inv = stats.tile([P, 1], F32, tag="inv")
                prev_inv_inst = nc.vector.reciprocal(out=inv[:], in_=xs[:])

            # quantize in halves: the transposes of half A start while
            # half B is still quantizing on ACT
            xq_parts = []
            for h in range(2):
                xq_h = xqpool.tile([P, KH], FP8, tag=f"xq{h}")
                nc.scalar.activation(
                    out=xq_h[:], in_=xt[:, h * KH:(h + 1) * KH],
                    func=mybir.ActivationFunctionType.Copy, scale=inv[:],
                )
                xq_parts.append(xq_h)

            def xq_chunk(ks):
                return xq_parts[ks // 16][:, (ks % 16) * P:(ks % 16 + 1) * P]

            # transpose x_q into [K, M] layout via PE transpose matmuls;
            # evict each 8-ksub group right after its matmuls so the GEMM's
            # j=0..3 can start as soon as the first group lands in SBUF
            xqt_groups = []
            for half in range(2):
                if USE_IS_TRANSPOSE:
                    # fp8 transpose mode writes elements on a 2-byte step
                    ptr = psum_tr.tile([P, 16, 2 * P], FP8, tag="ptr")
                    ptr_view = ptr[:, :, 0:2 * P:2]
                else:
                    ptr = psum_tr.tile([P, 16, P], F32, tag="ptr")
                    ptr_view = ptr[:]
                for g in range(2):
                    for i in range(8):
                        nc.tensor.matmul(
                            out=ptr_view[:, 8 * g + i, :],
                            lhsT=xq_chunk(half * 16 + 8 * g + i),
                            rhs=ident[:],
                            start=True, stop=True,
                            is_transpose=USE_IS_TRANSPOSE,
                        )
                    xqt_g = xqtpool.tile([P, 8, P], FP8, tag=f"xqt{2 * half + g}")
                    xqt_groups.append(xqt_g)
                    nc.scalar.copy(out=xqt_g[:], in_=ptr_view[:, 8 * g:8 * g + 8, :])
            xqt_tiles.append(xqt_groups)

            if mt >= 1:
                gemm_block(mt - 1, 1, phase1=True)
            gemm_block(mt, 0, phase1=True)

        gemm_block(M_TILES - 1, 1, phase1=True)

        # ---- phase 2: pure fp8 DoubleRow GEMM over the remaining N-blocks
        for nb in range(NB_PHASE1, N_BLKS):
            if nb + 1 < N_BLKS:
                issue_wslab(nb + 1, nc.sync)
            for mt in range(M_TILES):
                gemm_block(mt, nb)

    nc.compile()
    return nc


def _get_program():
    if "nc" not in _PROGRAM_CACHE:
        _PROGRAM_CACHE["nc"] = _build_program()
    return _PROGRAM_CACHE["nc"]


def _run_sharded(x, weight, weight_scales, bias, trace=False):
    x = np.asarray(x).astype(ml_dtypes.bfloat16, copy=False)
    weight = np.asarray(weight, dtype=np.float32)
    weight_scales = np.asarray(weight_scales, dtype=np.float32)
    bias = np.asarray(bias, dtype=np.float32)

    # host-side sharding / layout only:
    # wt[nb, p, ksub, n] = weight[nb*512 + n, ksub*128 + p], re-encoded to
    # fp8 e4m3 (lossless: the reference weights are fp8-round-tripped values)
    wt = np.ascontiguousarray(
        weight.T.reshape(K_SUBS, P, N_BLKS, N_BLK).transpose(2, 1, 0, 3)
    ).astype(ml_dtypes.float8_e4m3)
    in_maps = []
    for c in range(NCORES):
        in_maps.append({
            "x": np.ascontiguousarray(x[c * M_SHARD:(c + 1) * M_SHARD]),
            "wt": wt,
            "ws": weight_scales,
            "bias": bias,
        })

    nc = _get_program()
    res = run_bass_kernel_spmd(nc, in_maps, core_ids=list(range(NCORES)), trace=trace)
    out = np.concatenate([res.results[c]["out"] for c in range(NCORES)], axis=0)
    return out, res.exec_time_ns


def kernel(x, weight, weight_scales, bias):
    out, _ = _run_sharded(x, weight, weight_scales, bias,
                          trace=bool(os.environ.get("KERNEL_TRACE")))
    return out
